# revision 2
# baseline (speedup 1.0000x reference)
"""V4: class-sliced stage A + node-partitioned stage B, batched table rows.

Core g (of 8) owns:
  - stage A: class slices [g*CSH,(g+1)*CSH) in BOTH table halves; computes
    class-mean rows [class, B*H] via indicator matmuls from host-presorted
    member streams; two 8-way AllGathers assemble tableL/tableH (each
    [CPAD/2, B*H] bf16, < 32768 rows so int16 gather indices work).
  - stage B: nodes [g*NS,(g+1)*NS) for ALL batches; per edge one dma_gather
    of a 512B row (all 4 batches), indicator-weight matmuls (invn folded into
    host indw) accumulate ctx per 128-node block; emb added via identity
    matmul; fused LayerNorm MLP identical to the V3 scheme.
Low-half gathers overlap the second AllGather.
"""

import numpy as np

H = 64
LN_EPS = 1e-5
TG = 1024


def _ru(x, m):
    return (x + m - 1) // m * m


def _wrap16(idx):
    n = len(idx)
    n16 = _ru(n, 16)
    a = np.full(n16, -1, dtype=np.int16)
    a[:n] = idx
    a = a.reshape(n16 // 16, 16).T
    return np.tile(a, (8, 1)).copy()


def make_cfg(B, N, C, E):
    assert B == 4
    cfg = dict(B=B, N=N, C=C, E=E, n_cores=8)
    cfg["NPAD"] = _ru(N, 8 * 128)
    cfg["NS"] = cfg["NPAD"] // 8          # nodes per core
    cfg["NBLK"] = cfg["NS"] // 128
    cfg["CPAD"] = _ru(C, 16 * 128)
    cfg["CSH"] = cfg["CPAD"] // 16        # classes per (core, half)
    cfg["CBH"] = cfg["CSH"] // 128        # class blocks per (core, half)
    cfg["CHALF"] = cfg["CPAD"] // 2
    assert cfg["CHALF"] <= 32767
    cfg["GROUP"] = 512
    cfg["BH"] = B * H                     # 256
    return cfg


def host_prep(cfg, inputs):
    B, N, CPAD, CSH, CBH = cfg["B"], cfg["N"], cfg["CPAD"], cfg["CSH"], cfg["CBH"]
    NS, NBLK, CHALF = cfg["NS"], cfg["NBLK"], cfg["CHALF"]
    n_cores = cfg["n_cores"]
    c2n_row = np.asarray(inputs["c2n_row"]).astype(np.int64)
    c2n_col = np.asarray(inputs["c2n_col"]).astype(np.int64)
    n2c_row = np.asarray(inputs["n2c_row"]).astype(np.int64)
    n2c_col = np.asarray(inputs["n2c_col"]).astype(np.int64)

    cnt_c = np.bincount(c2n_row, minlength=CPAD).astype(np.float32)
    invc = (1.0 / np.maximum(cnt_c, 1.0)).astype(np.float32)
    cnt_n = np.bincount(n2c_row, minlength=cfg["NPAD"]).astype(np.float32)
    invn = (1.0 / np.maximum(cnt_n, 1.0)).astype(np.float32)

    # ---------------- stage A: member streams per (core, half) ----------
    # membership edges: node c2n_col[i] contributes to class c2n_row[i]
    order = np.argsort(c2n_row, kind="stable")
    mcls = c2n_row[order]           # sorted classes
    mnode = c2n_col[order]
    # core/half/block of each member
    mhalf = (mcls >= CHALF).astype(np.int64)
    mloc = mcls - mhalf * CHALF     # class index within half
    mcore = mloc // CSH
    mblk = (mloc % CSH) // 128      # local block within (core, half)
    # caps per (half, block) maxed over cores
    capsA = np.zeros((2, CBH), dtype=np.int64)
    cntA = np.zeros((n_cores, 2, CBH), dtype=np.int64)
    np.add.at(cntA, (mcore, mhalf, mblk), 1)
    capsA = _ru(np.maximum(cntA.max(axis=0), 1), 128)
    TAH = [int(capsA[h].sum()) for h in (0, 1)]
    TAH = [_ru(t, TG) for t in TAH]
    cfg["TAH"] = TAH
    cfg["TA"] = TAH[0] + TAH[1]
    schedA = []                       # per 128-chunk: (half, local block) or (h,-1)
    for h in (0, 1):
        nchunk = 0
        for j in range(CBH):
            for _ in range(capsA[h, j] // 128):
                schedA.append((h, j))
                nchunk += 1
        for _ in range(TAH[h] // 128 - nchunk):
            schedA.append((h, -1))

    # per-core stage A padded slots
    preA = []
    for g in range(n_cores):
        rowsrc = np.full(cfg["TA"], -1, dtype=np.int64)
        segA = np.full(cfg["TA"], 255, dtype=np.float32)
        base = 0
        for h in (0, 1):
            for j in range(CBH):
                m = (mcore == g) & (mhalf == h) & (mblk == j)
                nm = int(m.sum())
                rowsrc[base:base + nm] = np.nonzero(m)[0]
                segA[base:base + nm] = (mloc[m] % CSH) % 128
                base += capsA[h, j]
            base = TAH[0]
        import ml_dtypes
        bf = ml_dtypes.bfloat16
        segA_w = segA.reshape(-1, 128).T.copy().astype(bf)  # [128, TA/128]
        # invc columns [128, 2*CBH]
        ivc = np.zeros((128, 2 * CBH), dtype=np.float32)
        for h in (0, 1):
            for j in range(CBH):
                cbase = h * CHALF + g * CSH + j * 128
                ivc[:, h * CBH + j] = invc[cbase:cbase + 128]
        preA.append(dict(rowsrcA=rowsrc, segA=segA_w, invc_tok=ivc,
                         mnode=mnode))

    # ---------------- stage B: edge streams per core ---------------------
    ecore = n2c_row // NS
    ehalf = (n2c_col >= CHALF).astype(np.int64)
    erow = n2c_col - ehalf * CHALF    # gather row within half table
    edstl = n2c_row - ecore * NS      # local dst node
    enb = edstl // 128
    cntB = np.zeros((n_cores, 2, NBLK), dtype=np.int64)
    np.add.at(cntB, (ecore, ehalf, enb), 1)
    capsB = _ru(np.maximum(cntB.max(axis=0), 1), 128)   # [2, NBLK]
    TBH_ = [int(capsB[h].sum()) for h in (0, 1)]
    TBH_ = [_ru(t, TG) for t in TBH_]
    cfg["TBL"], cfg["TBHI"] = TBH_[0], TBH_[1]
    cfg["TB"] = TBH_[0] + TBH_[1]
    schedB = []
    for h in (0, 1):
        nchunk = 0
        for nb in range(NBLK):
            for _ in range(capsB[h, nb] // 128):
                schedB.append((h, nb))
                nchunk += 1
        for _ in range(TBH_[h] // 128 - nchunk):
            schedB.append((h, -1))

    preB = []
    import ml_dtypes
    f8 = ml_dtypes.float8_e4m3fn
    for g in range(n_cores):
        gidx = ((np.arange(cfg["TB"], dtype=np.int64) * 97) % 64)
        iw = np.zeros((cfg["TB"], 128), dtype=np.float32)
        base = 0
        for h in (0, 1):
            for nb in range(NBLK):
                m = (ecore == g) & (ehalf == h) & (enb == nb)
                nm = int(m.sum())
                gidx[base:base + nm] = erow[m]
                iw[np.arange(base, base + nm), edstl[m] % 128] = 1.0
                base += capsB[h, nb]
            base = TBH_[0]
        iw_w = np.ascontiguousarray(
            iw.astype(f8).reshape(-1, 128, 128).transpose(1, 0, 2))
        ivn = np.zeros((128, NBLK), dtype=np.float32)
        n0 = g * NS
        ivn[:, :] = invn[n0:n0 + NS].reshape(NBLK, 128).T
        deg = np.maximum(cnt_n[n0:n0 + NS], 1.0)
        preB.append(dict(gidxB=_wrap16(gidx), indwB=iw_w, invn_tok=ivn,
                         deg=deg))

    meta = dict(schedA=schedA, schedB=schedB)
    return preA, preB, meta


def weight_tensors(inputs):
    import ml_dtypes
    bf = ml_dtypes.bfloat16
    W1 = np.asarray(inputs["W1"], dtype=np.float32)
    b1 = np.asarray(inputs["b1"], dtype=np.float32)
    gamma = np.asarray(inputs["gamma"], dtype=np.float32)
    beta = np.asarray(inputs["beta"], dtype=np.float32)
    W2 = np.asarray(inputs["W2"], dtype=np.float32)
    w1blk = np.zeros((128, 128), dtype=np.float32)
    w1blk[:H, :H] = W1
    w1blk[H:, H:] = W1
    b1col = np.concatenate([b1, b1]).reshape(128, 1).astype(np.float32)
    stats = np.zeros((128, 2), dtype=np.float32)
    stats[:H, 0] = 1.0 / H
    stats[H:, 1] = 1.0 / H
    gamma2 = np.zeros((128, 128), dtype=np.float32)
    beta2 = np.zeros((128, 128), dtype=np.float32)
    for base in (0, 32, 64):
        gamma2[base, :H] = gamma
        gamma2[base + 1, H:] = gamma
        beta2[base, :H] = beta
        beta2[base + 1, H:] = beta
    w2col = np.zeros((128, 2), dtype=np.float32)
    w2col[:H, 0] = W2[:, 0]
    w2col[H:, 1] = W2[:, 0]
    iota = np.tile(np.arange(128, dtype=np.float32), (128, 1))
    beta2col = np.concatenate([beta, beta]).reshape(128, 1).astype(np.float32)
    return dict(
        w1blk=w1blk.astype(bf), b1col=b1col, beta2col=beta2col,
        stats_lhsT=stats.astype(bf), gamma2=gamma2.astype(bf),
        beta2=beta2.astype(bf), w2col=w2col.astype(bf),
        identd=np.eye(128, dtype=np.float32).astype(bf),
        iota_d=iota.astype(bf))


def build(cfg, meta, wvals):
    from concourse import bass, bacc, tile, mybir

    f32 = mybir.dt.float32
    bf16 = mybir.dt.bfloat16
    fp8 = mybir.dt.float8e4
    i16 = mybir.dt.int16
    AF = mybir.ActivationFunctionType
    ALU = mybir.AluOpType

    B, BH = cfg["B"], cfg["BH"]
    NBLK, CBH, CSH, CHALF = cfg["NBLK"], cfg["CBH"], cfg["CSH"], cfg["CHALF"]
    TA, TAH = cfg["TA"], cfg["TAH"]
    TB, TBL = cfg["TB"], cfg["TBL"]
    GROUP = cfg["GROUP"]
    n_cores = cfg["n_cores"]
    schedA, schedB = meta["schedA"], meta["schedB"]
    KC = TG // 128                       # 8 chunks per gather call
    NTOKC = NBLK * B * 128               # tokens per core
    assert NTOKC % GROUP == 0

    nc = bacc.Bacc("TRN2", target_bir_lowering=False, debug=False,
                   num_devices=n_cores, num_swdge_queues=4)

    embA = nc.dram_tensor("embA", [128, TA // 128, BH], bf16,
                          kind="ExternalInput")
    segA = nc.dram_tensor("segA", [128, TA // 128], bf16, kind="ExternalInput")
    embN = nc.dram_tensor("embN", [128, NBLK, BH], bf16, kind="ExternalInput")
    gidxB = nc.dram_tensor("gidxB", [128, TB // 16], i16, kind="ExternalInput")
    indwB = nc.dram_tensor("indwB", [128, TB // 128, 128], fp8,
                           kind="ExternalInput")
    invc_tok = nc.dram_tensor("invc_tok", [128, 2 * CBH], f32,
                              kind="ExternalInput")
    iota_d = nc.dram_tensor("iota_d", [128, 128], bf16, kind="ExternalInput")
    w1blk = nc.dram_tensor("w1blk", [128, 128], bf16, kind="ExternalInput")
    b1col = nc.dram_tensor("b1col", [128, 1], f32, kind="ExternalInput")
    stats_lhsT = nc.dram_tensor("stats_lhsT", [128, 2], bf16,
                                kind="ExternalInput")
    gamma2 = nc.dram_tensor("gamma2", [128, 128], bf16, kind="ExternalInput")
    beta2 = nc.dram_tensor("beta2", [128, 128], bf16, kind="ExternalInput")
    w2col = nc.dram_tensor("w2col", [128, 2], bf16, kind="ExternalInput")
    identd = nc.dram_tensor("identd", [128, 128], bf16, kind="ExternalInput")
    beta2col = nc.dram_tensor("beta2col", [128, 1], f32,
                              kind="ExternalInput")
    out = nc.dram_tensor("out", [NTOKC // GROUP, GROUP], f32,
                         kind="ExternalOutput")

    chslice = nc.dram_tensor("chslice", [2 * CSH, BH], fp8)
    tableL = nc.dram_tensor("tableL", [CHALF, BH], fp8)
    tableH = nc.dram_tensor("tableH", [CHALF, BH], fp8)
    invn_tok = nc.dram_tensor("invn_tok", [128, NBLK], f32,
                              kind="ExternalInput")

    b2v = float(wvals["b2"])
    groups8 = [list(range(n_cores))]

    with tile.TileContext(nc) as tc:
        AKC = 2 * KC  # embA DMA chunk: 2048 rows
        with tc.tile_pool(name="pw", bufs=1) as pw:
            w1_s = pw.tile([128, 128], bf16)
            b1_s = pw.tile([128, 1], f32)
            st_s = pw.tile([128, 2], bf16)
            ga_s = pw.tile([128, 128], bf16)
            be_s = pw.tile([128, 128], bf16)
            w2_s = pw.tile([128, 2], bf16)
            ident = pw.tile([128, 128], bf16)
            ones2 = pw.tile([128, GROUP], bf16)
            epsc = pw.tile([128, 1], f32)
            beta_c = pw.tile([128, 1], f32)
            ib_t = pw.tile([128, TB // 16], i16)
            iota_s = pw.tile([128, 128], bf16)
            sa = pw.tile([128, TA // 128], bf16)
            iv = pw.tile([128, 2 * CBH], f32)
            ivn_s = pw.tile([128, NBLK], f32)
            nc.scalar.dma_start(w1_s[:], w1blk[:, :])
            nc.scalar.dma_start(b1_s[:], b1col[:, :])
            nc.scalar.dma_start(st_s[:], stats_lhsT[:, :])
            nc.scalar.dma_start(ga_s[:], gamma2[:, :])
            nc.scalar.dma_start(be_s[:], beta2[:, :])
            nc.scalar.dma_start(w2_s[:], w2col[:, :])
            nc.scalar.dma_start(ident[:], identd[:, :])
            nc.scalar.dma_start(ib_t[:], gidxB[:, :])
            nc.scalar.dma_start(iota_s[:], iota_d[:, :])
            nc.scalar.dma_start(sa[:], segA[:, :])
            nc.scalar.dma_start(iv[:], invc_tok[:, :])
            nc.scalar.dma_start(ivn_s[:], invn_tok[:, :])
            nc.vector.memset(ones2[:], 1.0)
            nc.vector.memset(epsc[:], LN_EPS)
            nc.scalar.dma_start(beta_c[:], beta2col[:, :])

            # ================= stage A (per half) =================
            ck_base = [0, TAH[0] // 128]
            for h2 in (0, 1):
                with tc.tile_pool(name=f"pAg{h2}", bufs=4) as pAg, \
                     tc.tile_pool(name=f"pAn{h2}", bufs=1) as pAn, \
                     tc.tile_pool(name=f"pAp{h2}", bufs=2,
                                  space="PSUM") as pAp:
                    sbt = pAn.tile([128, CBH, BH], fp8, tag="sbt")
                    ck_lo = ck_base[h2]
                    ck_hi = ck_lo + TAH[h2] // 128
                    state = dict(tile=None, ind=None, pend=[], prev=None)

                    def flushA():
                        pend = state["pend"]
                        if not pend:
                            return
                        j = state["prev"]
                        ps = pAp.tile([128, BH], f32, tag="psA")
                        for i, (tl, ind_t, kk) in enumerate(pend):
                            nc.tensor.matmul(ps[:], ind_t[:, kk, :],
                                             tl[:, kk, :], start=(i == 0),
                                             stop=(i == len(pend) - 1))
                        col = h2 * CBH + j
                        ivb = (iv[:, col:col + 1]
                               .unsqueeze(2).broadcast_to([128, 1, BH]))
                        nc.vector.tensor_mul(sbt[:, j:j + 1, :],
                                             ps[:].unsqueeze(1), ivb)

                    for ck in range(ck_lo, ck_hi):
                        _h, j = schedA[ck]
                        if (ck - ck_lo) % AKC == 0:
                            n = min(AKC, ck_hi - ck)
                            state["tile"] = pAg.tile([128, AKC, BH], bf16,
                                                     tag="gA", name="gA")
                            eng = [nc.scalar, nc.sync][(ck // AKC) % 2]
                            eng.dma_start(
                                state["tile"][:, :n, :],
                                bass.AP(embA, ck * BH,
                                        [[TA // 128 * BH, 128], [BH, n],
                                         [1, BH]]))
                            state["ind"] = pAg.tile([128, AKC, 128], bf16,
                                                    tag="indA", name="indA")
                            nc.vector.tensor_tensor(
                                state["ind"][:, :n, :],
                                iota_s[:].unsqueeze(1)
                                .broadcast_to([128, n, 128]),
                                (sa[:, ck:ck + n]
                                 .unsqueeze(2).broadcast_to([128, n, 128])),
                                ALU.is_equal)
                        if j < 0:
                            continue
                        if j != state["prev"]:
                            flushA()
                            state["pend"] = []
                            state["prev"] = j
                        state["pend"].append(
                            (state["tile"], state["ind"], (ck - ck_lo) % AKC))
                    flushA()
                    for j0 in range(0, CBH, 5):
                        j1 = min(j0 + 5, CBH)
                        nc.sync.dma_start(
                            bass.AP(chslice, (h2 * CSH + j0 * 128) * BH,
                                    [[BH, 128], [128 * BH, j1 - j0],
                                     [1, BH]]),
                            sbt[:, j0:j1, :])
                tab = tableL if h2 == 0 else tableH
                nc.gpsimd.collective_compute(
                    "AllGather", mybir.AluOpType.bypass,
                    replica_groups=groups8,
                    ins=[bass.AP(chslice, h2 * CSH * BH,
                                 [[1, CSH * BH]]).opt()],
                    outs=[bass.AP(tab, 0, [[1, CHALF * BH]]).opt()])

            # ================= stage B =================
            capL = bass.AP(tableL, 0, [[BH, CHALF], [1, BH]])
            capH = bass.AP(tableH, 0, [[BH, CHALF], [1, BH]])

            # chunk lists per (half, nb)
            blk_chunks = {0: {}, 1: {}}
            for ck, (h, nb) in enumerate(schedB):
                if nb >= 0:
                    blk_chunks[h].setdefault(nb, []).append(ck)

            gtiles = {}

            with tc.tile_pool(name="pm", bufs=2) as pm, \
                 tc.tile_pool(name="pBg", bufs=8) as pBg, \
                 tc.tile_pool(name="pctx", bufs=1) as pctx, \
                 tc.tile_pool(name="pp", bufs=2, space="PSUM") as pp, \
                 tc.tile_pool(name="pp1", bufs=1, space="PSUM") as pp1, \
                 tc.tile_pool(name="ppB", bufs=2, space="PSUM") as ppB:

                ctxL = pctx.tile([128, NBLK, BH], bf16)

                def get_gtile(call):
                    if call not in gtiles:
                        src = capL if call < TBL // TG else capH
                        t = pBg.tile([128, KC, BH], fp8, tag="gB", bufs=10,
                                     name="gB")
                        nc.gpsimd.dma_gather(
                            t[:], src,
                            ib_t[:, call * TG // 16:(call + 1) * TG // 16],
                            TG, TG, BH, queue_num=call % 4)
                        iw = pBg.tile([128, KC, 128], fp8, tag="iwB", bufs=10,
                                      name="iwB")
                        nc.sync.dma_start(
                            iw[:],
                            bass.AP(indwB, call * KC * 128,
                                    [[TB, 128], [128, KC], [1, 128]]))
                        gtiles[call] = (t, iw)
                    return gtiles[call]

                # ---- LOW pass: ctxL per block (incl. emb*deg via ident)
                lastL = {nb: max(cks) // KC
                         for nb, cks in blk_chunks[0].items()}
                ncall_L = TBL // TG
                nbdone = 0
                etL = {}
                NES = 32
                for cg in range(0, ncall_L, 8):
                    for call in range(cg, min(cg + 8, ncall_L)):
                        get_gtile(call)
                    cov = min(cg + 8, ncall_L) - 1
                    while nbdone < NBLK and lastL.get(nbdone, -1) <= cov:
                        nb = nbdone
                        es = nb // NES
                        if es not in etL:
                            ne = min(NES, NBLK - es * NES)
                            etl = pm.tile([128, NES, BH], bf16, tag="embL",
                                          bufs=2, name="etl")
                            nc.scalar.dma_start(
                                etl[:, :ne, :],
                                bass.AP(embN, es * NES * BH,
                                        [[NBLK * BH, 128], [BH, ne],
                                         [1, BH]]))
                            etL[es] = etl
                        cks = blk_chunks[0].get(nb, [])
                        ps = ppB.tile([128, BH], f32, tag="psB")
                        nc.tensor.matmul(ps[:], ident[:],
                                         etL[es][:, nb % NES, :],
                                         start=True, stop=False)
                        for i, ck in enumerate(cks):
                            tl, iw = get_gtile(ck // KC)
                            nc.tensor.matmul(ps[:], iw[:, ck % KC, :],
                                             tl[:, ck % KC, :],
                                             start=False,
                                             stop=(i == len(cks) - 1))
                        nc.scalar.copy(ctxL[:, nb:nb + 1, :],
                                       ps[:].unsqueeze(1))
                        nbdone += 1

                # ---- HIGH pass + MLP, slabs of SLABB blocks
                SLABB = 24
                lastH = {nb: (max(cks) - TBL // 128) // KC
                         for nb, cks in blk_chunks[1].items()}
                ncall_H = (TB - TBL) // TG
                gpc = GROUP // 128
                nb0 = 0
                issuedH = -1
                while nb0 < NBLK:
                    nbs = min(SLABB, NBLK - nb0)
                    t0 = nb0 * B * 128           # token base
                    xt = pm.tile([128, SLABB, BH], bf16, tag="xm", bufs=3)
                    jdone = 0
                    while jdone < nbs:
                        if issuedH < ncall_H - 1:
                            hi = min(issuedH + 8, ncall_H - 1)
                            for call in range(issuedH + 1, hi + 1):
                                get_gtile(ncall_L + call)
                            issuedH = hi
                        while jdone < nbs and \
                                lastH.get(nb0 + jdone, -1) <= issuedH:
                            j = jdone
                            nb = nb0 + j
                            cks = blk_chunks[1].get(nb, [])
                            ps = ppB.tile([128, BH], f32, tag="psB")
                            nc.tensor.matmul(ps[:], ident[:],
                                             ctxL[:, nb, :],
                                             start=True, stop=False)
                            for i, ck in enumerate(cks):
                                call = ncall_L + (ck - TBL // 128) // KC
                                tl, iw = get_gtile(call)
                                kk = (ck - TBL // 128) % KC
                                nc.tensor.matmul(ps[:], iw[:, kk, :],
                                                 tl[:, kk, :],
                                                 start=False,
                                                 stop=(i == len(cks) - 1))
                            nc.vector.tensor_scalar(
                                xt[:, j:j + 1, :], ps[:].unsqueeze(1),
                                ivn_s[:, nb:nb + 1], None,
                                ALU.mult, ALU.bypass)
                            jdone += 1

                    # ---- MLP on this slab
                    ntok = nbs * B * 128
                    npr = ntok // (2 * GROUP)
                    for pg0 in range(0, npr, 3):
                        prs = list(range(pg0, min(pg0 + 3, npr)))
                        pst8 = pp1.tile([128, GROUP], f32, tag="pst8")
                        psq8 = pp1.tile([128, GROUP], f32, tag="psq8")
                        h1s = {}
                        for jl, pr in enumerate(prs):
                            xT = pp.tile([128, GROUP], bf16, tag="mmp")
                            for jj in range(gpc):
                                c0 = 2 * (pr * gpc + jj)
                                nc.tensor.transpose(
                                    xT[:, jj * 128:(jj + 1) * 128],
                                    xt[:, c0 // B,
                                       (c0 % B) * H:(c0 % B) * H + 2 * H]
                                    .unsqueeze(1), ident[:])
                            xT_sb = pm.tile([128, GROUP], bf16, tag="xTsb",
                                            bufs=3)
                            nc.scalar.copy(xT_sb[:], xT[:])
                            ph = pp.tile([128, GROUP], f32, tag="mmp")
                            nc.tensor.matmul(ph[:], w1_s[:], xT_sb[:])
                            h1 = pm.tile([128, GROUP], bf16, tag="h1",
                                         bufs=6)
                            sq = pm.tile([128, GROUP], bf16, tag="sq",
                                         bufs=3)
                            nc.vector.tensor_scalar(
                                h1[:], ph[:], b1_s[:], None, ALU.add,
                                ALU.bypass)
                            nc.scalar.activation(sq[:], h1[:], AF.Square)
                            nc.tensor.matmul(
                                pst8[32 * jl:32 * jl + 2, :], st_s[:], h1[:])
                            nc.tensor.matmul(
                                psq8[32 * jl:32 * jl + 2, :], st_s[:], sq[:])
                            h1s[pr] = h1
                        nrow = 32 * (len(prs) - 1) + 2
                        sm8 = pm.tile([128, GROUP], f32, tag="sm8", bufs=2)
                        var8 = pm.tile([128, GROUP], f32, tag="var8", bufs=2)
                        sd8 = pm.tile([128, GROUP], f32, tag="sd8", bufs=2)
                        rstd8 = pm.tile([128, GROUP], f32, tag="rstd8",
                                        bufs=2)
                        rstd8_bf = pm.tile([128, GROUP], bf16, tag="rstd8b",
                                           bufs=2)
                        affr8 = pm.tile([128, GROUP], bf16, tag="affr8",
                                        bufs=2)
                        nc.scalar.copy(sm8[:nrow, :], pst8[:nrow, :])
                        nc.vector.scalar_tensor_tensor(
                            var8[:nrow, :], sm8[:nrow, :], -1.0,
                            sm8[:nrow, :], ALU.mult, ALU.mult)
                        nc.vector.scalar_tensor_tensor(
                            var8[:nrow, :], psq8[:nrow, :], 1.0,
                            var8[:nrow, :], ALU.mult, ALU.add)
                        nc.scalar.activation(sd8[:nrow, :], var8[:nrow, :],
                                             AF.Sqrt, bias=epsc[:nrow, :],
                                             scale=1.0)
                        nc.vector.reciprocal_approx_fast(rstd8[:nrow, :],
                                                         sd8[:nrow, :])
                        nc.scalar.copy(rstd8_bf[:nrow, :], rstd8[:nrow, :])
                        nc.vector.scalar_tensor_tensor(
                            affr8[:nrow, :], sm8[:nrow, :], -1.0,
                            rstd8[:nrow, :], ALU.mult, ALU.mult)
                        for jl, pr in enumerate(prs):
                            h1 = h1s[pr]
                            pscale = pp1.tile([128, GROUP], f32,
                                              tag="pscale", bufs=2)
                            poff = pp1.tile([128, GROUP], f32, tag="pscale",
                                            bufs=2, name="poff")
                            sl = slice(32 * jl, 32 * jl + 2)
                            nc.tensor.matmul(
                                pscale[:], ga_s[sl, :], rstd8_bf[sl, :])
                            nc.tensor.matmul(poff[:], ga_s[sl, :],
                                             affr8[sl, :])
                            t1t = pm.tile([128, GROUP], f32, tag="t1t")
                            h3 = pm.tile([128, GROUP], bf16, tag="h3")
                            nc.vector.tensor_mul(t1t[:], h1[:], pscale[:])
                            nc.vector.tensor_add(t1t[:], t1t[:], poff[:])
                            nc.scalar.activation(h3[:], t1t[:], AF.Relu,
                                                 bias=beta_c[:], scale=1.0)
                            pL2 = pp1.tile([2, GROUP], f32, tag="pscale", bufs=2,
                                           name="pL2")
                            nc.tensor.matmul(pL2[:], w2_s[:], h3[:])
                            lgs = pm.tile([2, GROUP], f32, tag="lgs",
                                          bufs=3)
                            nc.vector.tensor_copy(lgs[:], pL2[:])
                            nc.sync.dma_start(
                                bass.AP(out, t0 + pr * 2 * GROUP,
                                        [[GROUP, 2], [1, GROUP]]),
                                lgs[:])
                    nb0 += nbs

    nc.compile()
    return nc


def build_in_maps(cfg, inputs, preA, preB, wts):
    import ml_dtypes
    bf = ml_dtypes.bfloat16
    emb_full = np.asarray(inputs["embedding"], dtype=np.float32)
    B, N, NS, NBLK, BH, TA = (cfg["B"], cfg["N"], cfg["NS"], cfg["NBLK"],
                              cfg["BH"], cfg["TA"])
    # [N, B*H] view of the embedding
    embT = np.ascontiguousarray(emb_full.transpose(1, 0, 2).reshape(N, BH))
    in_maps = []
    for g in range(cfg["n_cores"]):
        dA, dB = preA[g], preB[g]
        embA_a = np.zeros((TA, BH), dtype=bf)
        valid = dA["rowsrcA"] >= 0
        src_nodes = dA["mnode"][dA["rowsrcA"][valid]]
        embA_a[valid] = embT[src_nodes].astype(bf)
        embA_a = np.ascontiguousarray(
            embA_a.reshape(-1, 128, BH).transpose(1, 0, 2))
        embN_a = np.zeros((NS, BH), dtype=bf)
        n0 = g * NS
        n1 = min(n0 + NS, N)
        deg = preB[g]["deg"][:n1 - n0]
        embN_a[:n1 - n0] = (embT[n0:n1] * deg[:, None]).astype(bf)
        embN_a = np.ascontiguousarray(
            embN_a.reshape(-1, 128, BH).transpose(1, 0, 2))
        m = dict(embA=embA_a, segA=dA["segA"], invc_tok=dA["invc_tok"],
                 embN=embN_a, gidxB=dB["gidxB"], indwB=dB["indwB"],
                 invn_tok=dB["invn_tok"], **wts)
        in_maps.append(m)
    return in_maps


def assemble_out(cfg, results, b2v=0.0):
    B, N, NS, NBLK, GROUP = (cfg["B"], cfg["N"], cfg["NS"], cfg["NBLK"],
                             cfg["GROUP"])
    out = np.empty((B, N), dtype=np.float32)
    for g in range(cfg["n_cores"]):
        a = np.asarray(results[g]["out"]).reshape(-1, 2, 4, 128) + b2v
        toks = a.transpose(0, 2, 1, 3).reshape(-1)   # token-major
        # token t = (nb*B + b)*128 + p ; node = g*NS + nb*128 + p
        t = toks.reshape(NBLK, B, 128)
        n0 = g * NS
        n1 = min(n0 + NS, N)
        for b in range(B):
            out[b, n0:n1] = t[:, b, :].reshape(-1)[:n1 - n0]
    return out


def kernel(**inputs):
    emb = np.asarray(inputs["embedding"])
    B, N, _ = emb.shape
    C = int(inputs["num_classes"])
    E = len(np.asarray(inputs["n2c_row"]))
    cfg = make_cfg(B, N, C, E)
    preA, preB, meta = host_prep(cfg, inputs)
    wts = weight_tensors(inputs)
    wvals = dict(b2=float(np.asarray(inputs["b2"]).reshape(-1)[0]))
    nc = build(cfg, meta, wvals)
    in_maps = build_in_maps(cfg, inputs, preA, preB, wts)
    from concourse.bass_utils import run_bass_kernel_spmd
    res = run_bass_kernel_spmd(nc, in_maps,
                               core_ids=list(range(cfg["n_cores"])))
    return assemble_out(cfg, res.results, wvals["b2"])


# revision 3
# speedup vs baseline: 1.0607x; 1.0607x over previous
"""V4: class-sliced stage A + node-partitioned stage B, batched table rows.

Core g (of 8) owns:
  - stage A: class slices [g*CSH,(g+1)*CSH) in BOTH table halves; computes
    class-mean rows [class, B*H] via indicator matmuls from host-presorted
    member streams; two 8-way AllGathers assemble tableL/tableH (each
    [CPAD/2, B*H] bf16, < 32768 rows so int16 gather indices work).
  - stage B: nodes [g*NS,(g+1)*NS) for ALL batches; per edge one dma_gather
    of a 512B row (all 4 batches), indicator-weight matmuls (invn folded into
    host indw) accumulate ctx per 128-node block; emb added via identity
    matmul; fused LayerNorm MLP identical to the V3 scheme.
Low-half gathers overlap the second AllGather.
"""

import numpy as np

H = 64
LN_EPS = 1e-5
TG = 1024


def _ru(x, m):
    return (x + m - 1) // m * m


def _wrap16(idx):
    n = len(idx)
    n16 = _ru(n, 16)
    a = np.full(n16, -1, dtype=np.int16)
    a[:n] = idx
    a = a.reshape(n16 // 16, 16).T
    return np.tile(a, (8, 1)).copy()


def make_cfg(B, N, C, E):
    assert B == 4
    cfg = dict(B=B, N=N, C=C, E=E, n_cores=8)
    cfg["NPAD"] = _ru(N, 8 * 128)
    cfg["NS"] = cfg["NPAD"] // 8          # nodes per core
    cfg["NBLK"] = cfg["NS"] // 128
    cfg["CPAD"] = _ru(C, 16 * 128)
    cfg["CSH"] = cfg["CPAD"] // 16        # classes per (core, half)
    cfg["CBH"] = cfg["CSH"] // 128        # class blocks per (core, half)
    cfg["CHALF"] = cfg["CPAD"] // 2
    assert cfg["CHALF"] <= 32767
    cfg["GROUP"] = 512
    cfg["BH"] = B * H                     # 256
    return cfg


def host_prep(cfg, inputs):
    B, N, CPAD, CSH, CBH = cfg["B"], cfg["N"], cfg["CPAD"], cfg["CSH"], cfg["CBH"]
    NS, NBLK, CHALF = cfg["NS"], cfg["NBLK"], cfg["CHALF"]
    n_cores = cfg["n_cores"]
    c2n_row = np.asarray(inputs["c2n_row"]).astype(np.int64)
    c2n_col = np.asarray(inputs["c2n_col"]).astype(np.int64)
    n2c_row = np.asarray(inputs["n2c_row"]).astype(np.int64)
    n2c_col = np.asarray(inputs["n2c_col"]).astype(np.int64)

    cnt_c = np.bincount(c2n_row, minlength=CPAD).astype(np.float32)
    invc = (1.0 / np.maximum(cnt_c, 1.0)).astype(np.float32)
    cnt_n = np.bincount(n2c_row, minlength=cfg["NPAD"]).astype(np.float32)
    invn = (1.0 / np.maximum(cnt_n, 1.0)).astype(np.float32)

    # ---------------- stage A: member streams per (core, half) ----------
    # membership edges: node c2n_col[i] contributes to class c2n_row[i]
    order = np.argsort(c2n_row, kind="stable")
    mcls = c2n_row[order]           # sorted classes
    mnode = c2n_col[order]
    # core/half/block of each member
    mhalf = (mcls >= CHALF).astype(np.int64)
    mloc = mcls - mhalf * CHALF     # class index within half
    mcore = mloc // CSH
    mblk = (mloc % CSH) // 128      # local block within (core, half)
    # caps per (half, block) maxed over cores
    capsA = np.zeros((2, CBH), dtype=np.int64)
    cntA = np.zeros((n_cores, 2, CBH), dtype=np.int64)
    np.add.at(cntA, (mcore, mhalf, mblk), 1)
    capsA = _ru(np.maximum(cntA.max(axis=0), 1), 128)
    TAH = [int(capsA[h].sum()) for h in (0, 1)]
    TAH = [_ru(t, TG) for t in TAH]
    cfg["TAH"] = TAH
    cfg["TA"] = TAH[0] + TAH[1]
    schedA = []                       # per 128-chunk: (half, local block) or (h,-1)
    for h in (0, 1):
        nchunk = 0
        for j in range(CBH):
            for _ in range(capsA[h, j] // 128):
                schedA.append((h, j))
                nchunk += 1
        for _ in range(TAH[h] // 128 - nchunk):
            schedA.append((h, -1))

    # per-core stage A padded slots
    preA = []
    for g in range(n_cores):
        rowsrc = np.full(cfg["TA"], -1, dtype=np.int64)
        segA = np.full(cfg["TA"], 255, dtype=np.float32)
        base = 0
        for h in (0, 1):
            for j in range(CBH):
                m = (mcore == g) & (mhalf == h) & (mblk == j)
                nm = int(m.sum())
                rowsrc[base:base + nm] = np.nonzero(m)[0]
                segA[base:base + nm] = (mloc[m] % CSH) % 128
                base += capsA[h, j]
            base = TAH[0]
        import ml_dtypes
        bf = ml_dtypes.bfloat16
        import ml_dtypes as _md
        f8a = _md.float8_e4m3fn
        ia = np.zeros((cfg["TA"], 128), dtype=np.float32)
        vv = segA < 255
        ia[np.nonzero(vv)[0], segA[vv].astype(np.int64)] = 1.0
        indA_w = np.ascontiguousarray(
            ia.astype(f8a).reshape(-1, 128, 128).transpose(1, 0, 2))
        preA.append(dict(rowsrcA=rowsrc, indA=indA_w,
                         mnode=mnode, minvc=invc[mcls]))

    # ---------------- stage B: edge streams per core ---------------------
    ecore = n2c_row // NS
    ehalf = (n2c_col >= CHALF).astype(np.int64)
    erow = n2c_col - ehalf * CHALF    # gather row within half table
    edstl = n2c_row - ecore * NS      # local dst node
    enb = edstl // 128
    cntB = np.zeros((n_cores, 2, NBLK), dtype=np.int64)
    np.add.at(cntB, (ecore, ehalf, enb), 1)
    capsB = _ru(np.maximum(cntB.max(axis=0), 1), 128)   # [2, NBLK]
    TBH_ = [int(capsB[h].sum()) for h in (0, 1)]
    TBH_ = [_ru(t, TG) for t in TBH_]
    cfg["TBL"], cfg["TBHI"] = TBH_[0], TBH_[1]
    cfg["TB"] = TBH_[0] + TBH_[1]
    schedB = []
    for h in (0, 1):
        nchunk = 0
        for nb in range(NBLK):
            for _ in range(capsB[h, nb] // 128):
                schedB.append((h, nb))
                nchunk += 1
        for _ in range(TBH_[h] // 128 - nchunk):
            schedB.append((h, -1))

    preB = []
    import ml_dtypes
    f8 = ml_dtypes.float8_e4m3fn
    for g in range(n_cores):
        gidx = ((np.arange(cfg["TB"], dtype=np.int64) * 97) % 64)
        iw = np.zeros((cfg["TB"], 128), dtype=np.float32)
        base = 0
        for h in (0, 1):
            for nb in range(NBLK):
                m = (ecore == g) & (ehalf == h) & (enb == nb)
                nm = int(m.sum())
                gidx[base:base + nm] = erow[m]
                iw[np.arange(base, base + nm), edstl[m] % 128] = 1.0
                base += capsB[h, nb]
            base = TBH_[0]
        iw_w = np.ascontiguousarray(
            iw.astype(f8).reshape(-1, 128, 128).transpose(1, 0, 2))
        ivn = np.zeros((128, NBLK), dtype=np.float32)
        n0 = g * NS
        ivn[:, :] = invn[n0:n0 + NS].reshape(NBLK, 128).T
        deg = np.maximum(cnt_n[n0:n0 + NS], 1.0)
        preB.append(dict(gidxB=_wrap16(gidx), indwB=iw_w, invn_tok=ivn,
                         deg=deg))

    meta = dict(schedA=schedA, schedB=schedB)
    return preA, preB, meta


def weight_tensors(inputs):
    import ml_dtypes
    bf = ml_dtypes.bfloat16
    W1 = np.asarray(inputs["W1"], dtype=np.float32)
    b1 = np.asarray(inputs["b1"], dtype=np.float32)
    gamma = np.asarray(inputs["gamma"], dtype=np.float32)
    beta = np.asarray(inputs["beta"], dtype=np.float32)
    W2 = np.asarray(inputs["W2"], dtype=np.float32)
    w1blk = np.zeros((128, 128), dtype=np.float32)
    w1blk[:H, :H] = W1
    w1blk[H:, H:] = W1
    b1col = np.concatenate([b1, b1]).reshape(128, 1).astype(np.float32)
    stats = np.zeros((128, 2), dtype=np.float32)
    stats[:H, 0] = 1.0 / H
    stats[H:, 1] = 1.0 / H
    gamma2 = np.zeros((128, 128), dtype=np.float32)
    beta2 = np.zeros((128, 128), dtype=np.float32)
    for base in (0, 32, 64):
        gamma2[base, :H] = gamma
        gamma2[base + 1, H:] = gamma
        beta2[base, :H] = beta
        beta2[base + 1, H:] = beta
    w2col = np.zeros((128, 2), dtype=np.float32)
    w2col[:H, 0] = W2[:, 0]
    w2col[H:, 1] = W2[:, 0]
    iota = np.tile(np.arange(128, dtype=np.float32), (128, 1))
    beta2col = np.concatenate([beta, beta]).reshape(128, 1).astype(np.float32)
    return dict(
        w1blk=w1blk.astype(bf), b1col=b1col, beta2col=beta2col,
        stats_lhsT=stats.astype(bf), gamma2=gamma2.astype(bf),
        beta2=beta2.astype(bf), w2col=w2col.astype(bf),
        identd=np.eye(128, dtype=np.float32).astype(bf),
        iota_d=iota.astype(bf))


def build(cfg, meta, wvals):
    from concourse import bass, bacc, tile, mybir

    f32 = mybir.dt.float32
    bf16 = mybir.dt.bfloat16
    fp8 = mybir.dt.float8e4
    i16 = mybir.dt.int16
    AF = mybir.ActivationFunctionType
    ALU = mybir.AluOpType

    B, BH = cfg["B"], cfg["BH"]
    NBLK, CBH, CSH, CHALF = cfg["NBLK"], cfg["CBH"], cfg["CSH"], cfg["CHALF"]
    TA, TAH = cfg["TA"], cfg["TAH"]
    TB, TBL = cfg["TB"], cfg["TBL"]
    GROUP = cfg["GROUP"]
    n_cores = cfg["n_cores"]
    schedA, schedB = meta["schedA"], meta["schedB"]
    KC = TG // 128                       # 8 chunks per gather call
    NTOKC = NBLK * B * 128               # tokens per core
    assert NTOKC % GROUP == 0

    nc = bacc.Bacc("TRN2", target_bir_lowering=False, debug=False,
                   num_devices=n_cores, num_swdge_queues=4)

    embA = nc.dram_tensor("embA", [128, TA // 128, BH], bf16,
                          kind="ExternalInput")
    indA = nc.dram_tensor("indA", [128, TA // 128, 128], fp8,
                          kind="ExternalInput")
    embN = nc.dram_tensor("embN", [128, NBLK, BH], bf16, kind="ExternalInput")
    gidxB = nc.dram_tensor("gidxB", [128, TB // 16], i16, kind="ExternalInput")
    indwB = nc.dram_tensor("indwB", [128, TB // 128, 128], fp8,
                           kind="ExternalInput")
    iota_d = nc.dram_tensor("iota_d", [128, 128], bf16, kind="ExternalInput")
    w1blk = nc.dram_tensor("w1blk", [128, 128], bf16, kind="ExternalInput")
    b1col = nc.dram_tensor("b1col", [128, 1], f32, kind="ExternalInput")
    stats_lhsT = nc.dram_tensor("stats_lhsT", [128, 2], bf16,
                                kind="ExternalInput")
    gamma2 = nc.dram_tensor("gamma2", [128, 128], bf16, kind="ExternalInput")
    beta2 = nc.dram_tensor("beta2", [128, 128], bf16, kind="ExternalInput")
    w2col = nc.dram_tensor("w2col", [128, 2], bf16, kind="ExternalInput")
    identd = nc.dram_tensor("identd", [128, 128], bf16, kind="ExternalInput")
    beta2col = nc.dram_tensor("beta2col", [128, 1], f32,
                              kind="ExternalInput")
    out = nc.dram_tensor("out", [NTOKC // GROUP, GROUP], f32,
                         kind="ExternalOutput")

    chslice = nc.dram_tensor("chslice", [2 * CSH, BH], fp8)
    tableL = nc.dram_tensor("tableL", [CHALF, BH], fp8)
    tableH = nc.dram_tensor("tableH", [CHALF, BH], fp8)
    invn_tok = nc.dram_tensor("invn_tok", [128, NBLK], f32,
                              kind="ExternalInput")

    b2v = float(wvals["b2"])
    groups8 = [list(range(n_cores))]

    with tile.TileContext(nc) as tc:
        AKC = 2 * KC  # embA DMA chunk: 2048 rows
        with tc.tile_pool(name="pw", bufs=1) as pw:
            w1_s = pw.tile([128, 128], bf16)
            b1_s = pw.tile([128, 1], f32)
            st_s = pw.tile([128, 2], bf16)
            ga_s = pw.tile([128, 128], bf16)
            be_s = pw.tile([128, 128], bf16)
            w2_s = pw.tile([128, 2], bf16)
            ident = pw.tile([128, 128], bf16)
            ones2 = pw.tile([128, GROUP], bf16)
            epsc = pw.tile([128, 1], f32)
            beta_c = pw.tile([128, 1], f32)
            ib_t = pw.tile([128, TB // 16], i16)
            iota_s = pw.tile([128, 128], bf16)
            ivn_s = pw.tile([128, NBLK], f32)
            nc.scalar.dma_start(w1_s[:], w1blk[:, :])
            nc.scalar.dma_start(b1_s[:], b1col[:, :])
            nc.scalar.dma_start(st_s[:], stats_lhsT[:, :])
            nc.scalar.dma_start(ga_s[:], gamma2[:, :])
            nc.scalar.dma_start(be_s[:], beta2[:, :])
            nc.scalar.dma_start(w2_s[:], w2col[:, :])
            nc.scalar.dma_start(ident[:], identd[:, :])
            nc.scalar.dma_start(ib_t[:], gidxB[:, :])
            nc.scalar.dma_start(iota_s[:], iota_d[:, :])
            nc.scalar.dma_start(ivn_s[:], invn_tok[:, :])
            nc.vector.memset(ones2[:], 1.0)
            nc.vector.memset(epsc[:], LN_EPS)
            nc.scalar.dma_start(beta_c[:], beta2col[:, :])

            # ================= stage A (per half) =================
            ck_base = [0, TAH[0] // 128]
            for h2 in (0, 1):
                with tc.tile_pool(name=f"pAg{h2}", bufs=4) as pAg, \
                     tc.tile_pool(name=f"pAn{h2}", bufs=1) as pAn, \
                     tc.tile_pool(name=f"pAp{h2}", bufs=2,
                                  space="PSUM") as pAp:
                    sbt = pAn.tile([128, CBH, BH], fp8, tag="sbt")
                    ck_lo = ck_base[h2]
                    ck_hi = ck_lo + TAH[h2] // 128
                    state = dict(tile=None, ind=None, pend=[], prev=None)

                    def flushA():
                        pend = state["pend"]
                        if not pend:
                            return
                        j = state["prev"]
                        ps = pAp.tile([128, BH], f32, tag="psA")
                        for i, (tl, ind_t, kk) in enumerate(pend):
                            nc.tensor.matmul(ps[:], ind_t[:, kk, :],
                                             tl[:, kk, :], start=(i == 0),
                                             stop=(i == len(pend) - 1))
                        nc.scalar.copy(sbt[:, j:j + 1, :],
                                       ps[:].unsqueeze(1))

                    for ck in range(ck_lo, ck_hi):
                        _h, j = schedA[ck]
                        if (ck - ck_lo) % AKC == 0:
                            n = min(AKC, ck_hi - ck)
                            state["tile"] = pAg.tile([128, AKC, BH], bf16,
                                                     tag="gA", name="gA")
                            eng = [nc.scalar, nc.sync][(ck // AKC) % 2]
                            eng.dma_start(
                                state["tile"][:, :n, :],
                                bass.AP(embA, ck * BH,
                                        [[TA // 128 * BH, 128], [BH, n],
                                         [1, BH]]))
                            state["ind"] = pAg.tile([128, AKC, 128], fp8,
                                                    tag="indA", name="indAt")
                            eng2 = [nc.sync, nc.scalar][(ck // AKC) % 2]
                            eng2.dma_start(
                                state["ind"][:, :n, :],
                                bass.AP(indA, ck * 128,
                                        [[TA // 128 * 128, 128], [128, n],
                                         [1, 128]]))
                        if j < 0:
                            continue
                        if j != state["prev"]:
                            flushA()
                            state["pend"] = []
                            state["prev"] = j
                        state["pend"].append(
                            (state["tile"], state["ind"], (ck - ck_lo) % AKC))
                    flushA()
                    for j0 in range(0, CBH, 5):
                        j1 = min(j0 + 5, CBH)
                        nc.sync.dma_start(
                            bass.AP(chslice, (h2 * CSH + j0 * 128) * BH,
                                    [[BH, 128], [128 * BH, j1 - j0],
                                     [1, BH]]),
                            sbt[:, j0:j1, :])
                tab = tableL if h2 == 0 else tableH
                nc.gpsimd.collective_compute(
                    "AllGather", mybir.AluOpType.bypass,
                    replica_groups=groups8,
                    ins=[bass.AP(chslice, h2 * CSH * BH,
                                 [[1, CSH * BH]]).opt()],
                    outs=[bass.AP(tab, 0, [[1, CHALF * BH]]).opt()])

            # ================= stage B =================
            capL = bass.AP(tableL, 0, [[BH, CHALF], [1, BH]])
            capH = bass.AP(tableH, 0, [[BH, CHALF], [1, BH]])

            # chunk lists per (half, nb)
            blk_chunks = {0: {}, 1: {}}
            for ck, (h, nb) in enumerate(schedB):
                if nb >= 0:
                    blk_chunks[h].setdefault(nb, []).append(ck)

            gtiles = {}

            with tc.tile_pool(name="pm", bufs=2) as pm, \
                 tc.tile_pool(name="pBg", bufs=8) as pBg, \
                 tc.tile_pool(name="pctx", bufs=1) as pctx, \
                 tc.tile_pool(name="pp", bufs=2, space="PSUM") as pp, \
                 tc.tile_pool(name="pp1", bufs=1, space="PSUM") as pp1, \
                 tc.tile_pool(name="ppB", bufs=2, space="PSUM") as ppB:

                ctxL = pctx.tile([128, NBLK, BH], bf16)

                def get_gtile(call):
                    if call not in gtiles:
                        src = capL if call < TBL // TG else capH
                        t = pBg.tile([128, KC, BH], fp8, tag="gB", bufs=10,
                                     name="gB")
                        nc.gpsimd.dma_gather(
                            t[:], src,
                            ib_t[:, call * TG // 16:(call + 1) * TG // 16],
                            TG, TG, BH, queue_num=call % 4)
                        iw = pBg.tile([128, KC, 128], fp8, tag="iwB", bufs=10,
                                      name="iwB")
                        nc.sync.dma_start(
                            iw[:],
                            bass.AP(indwB, call * KC * 128,
                                    [[TB, 128], [128, KC], [1, 128]]))
                        gtiles[call] = (t, iw)
                    return gtiles[call]

                # ---- LOW pass: ctxL per block (incl. emb*deg via ident)
                lastL = {nb: max(cks) // KC
                         for nb, cks in blk_chunks[0].items()}
                ncall_L = TBL // TG
                nbdone = 0
                etL = {}
                NES = 32
                for cg in range(0, ncall_L, 8):
                    for call in range(cg, min(cg + 8, ncall_L)):
                        get_gtile(call)
                    cov = min(cg + 8, ncall_L) - 1
                    while nbdone < NBLK and lastL.get(nbdone, -1) <= cov:
                        nb = nbdone
                        es = nb // NES
                        if es not in etL:
                            ne = min(NES, NBLK - es * NES)
                            etl = pm.tile([128, NES, BH], bf16, tag="embL",
                                          bufs=2, name="etl")
                            nc.scalar.dma_start(
                                etl[:, :ne, :],
                                bass.AP(embN, es * NES * BH,
                                        [[NBLK * BH, 128], [BH, ne],
                                         [1, BH]]))
                            etL[es] = etl
                        cks = blk_chunks[0].get(nb, [])
                        ps = ppB.tile([128, BH], f32, tag="psB")
                        nc.tensor.matmul(ps[:], ident[:],
                                         etL[es][:, nb % NES, :],
                                         start=True, stop=False)
                        for i, ck in enumerate(cks):
                            tl, iw = get_gtile(ck // KC)
                            nc.tensor.matmul(ps[:], iw[:, ck % KC, :],
                                             tl[:, ck % KC, :],
                                             start=False,
                                             stop=(i == len(cks) - 1))
                        nc.scalar.copy(ctxL[:, nb:nb + 1, :],
                                       ps[:].unsqueeze(1))
                        nbdone += 1

                # ---- HIGH pass + MLP, slabs of SLABB blocks
                SLABB = 24
                lastH = {nb: (max(cks) - TBL // 128) // KC
                         for nb, cks in blk_chunks[1].items()}
                ncall_H = (TB - TBL) // TG
                gpc = GROUP // 128
                nb0 = 0
                issuedH = -1
                while nb0 < NBLK:
                    nbs = min(SLABB, NBLK - nb0)
                    t0 = nb0 * B * 128           # token base
                    xt = pm.tile([128, SLABB, BH], bf16, tag="xm", bufs=3)
                    jdone = 0
                    while jdone < nbs:
                        if issuedH < ncall_H - 1:
                            hi = min(issuedH + 8, ncall_H - 1)
                            for call in range(issuedH + 1, hi + 1):
                                get_gtile(ncall_L + call)
                            issuedH = hi
                        while jdone < nbs and \
                                lastH.get(nb0 + jdone, -1) <= issuedH:
                            j = jdone
                            nb = nb0 + j
                            cks = blk_chunks[1].get(nb, [])
                            ps = ppB.tile([128, BH], f32, tag="psB")
                            nc.tensor.matmul(ps[:], ident[:],
                                             ctxL[:, nb, :],
                                             start=True, stop=False)
                            for i, ck in enumerate(cks):
                                call = ncall_L + (ck - TBL // 128) // KC
                                tl, iw = get_gtile(call)
                                kk = (ck - TBL // 128) % KC
                                nc.tensor.matmul(ps[:], iw[:, kk, :],
                                                 tl[:, kk, :],
                                                 start=False,
                                                 stop=(i == len(cks) - 1))
                            nc.vector.tensor_scalar(
                                xt[:, j:j + 1, :], ps[:].unsqueeze(1),
                                ivn_s[:, nb:nb + 1], None,
                                ALU.mult, ALU.bypass)
                            jdone += 1

                    # ---- MLP on this slab
                    ntok = nbs * B * 128
                    npr = ntok // (2 * GROUP)
                    for pg0 in range(0, npr, 3):
                        prs = list(range(pg0, min(pg0 + 3, npr)))
                        pst8 = pp1.tile([128, GROUP], f32, tag="pst8")
                        psq8 = pp1.tile([128, GROUP], f32, tag="psq8")
                        h1s = {}
                        for jl, pr in enumerate(prs):
                            xT = pp.tile([128, GROUP], bf16, tag="mmp")
                            for jj in range(gpc):
                                c0 = 2 * (pr * gpc + jj)
                                nc.tensor.transpose(
                                    xT[:, jj * 128:(jj + 1) * 128],
                                    xt[:, c0 // B,
                                       (c0 % B) * H:(c0 % B) * H + 2 * H]
                                    .unsqueeze(1), ident[:])
                            xT_sb = pm.tile([128, GROUP], bf16, tag="xTsb",
                                            bufs=3)
                            nc.scalar.copy(xT_sb[:], xT[:])
                            ph = pp.tile([128, GROUP], f32, tag="mmp")
                            nc.tensor.matmul(ph[:], w1_s[:], xT_sb[:])
                            h1 = pm.tile([128, GROUP], bf16, tag="h1",
                                         bufs=6)
                            sq = pm.tile([128, GROUP], bf16, tag="sq",
                                         bufs=3)
                            nc.scalar.activation(h1[:], ph[:], AF.Identity,
                                                 bias=b1_s[:], scale=1.0)
                            nc.scalar.activation(sq[:], h1[:], AF.Square)
                            nc.tensor.matmul(
                                pst8[32 * jl:32 * jl + 2, :], st_s[:], h1[:])
                            nc.tensor.matmul(
                                psq8[32 * jl:32 * jl + 2, :], st_s[:], sq[:])
                            h1s[pr] = h1
                        nrow = 32 * (len(prs) - 1) + 2
                        sm8 = pm.tile([128, GROUP], f32, tag="sm8", bufs=2)
                        var8 = pm.tile([128, GROUP], f32, tag="var8", bufs=2)
                        sd8 = pm.tile([128, GROUP], f32, tag="sd8", bufs=2)
                        rstd8 = pm.tile([128, GROUP], f32, tag="rstd8",
                                        bufs=2)
                        rstd8_bf = pm.tile([128, GROUP], bf16, tag="rstd8b",
                                           bufs=2)
                        affr8 = pm.tile([128, GROUP], bf16, tag="affr8",
                                        bufs=2)
                        nc.scalar.copy(sm8[:nrow, :], pst8[:nrow, :])
                        nc.vector.scalar_tensor_tensor(
                            var8[:nrow, :], sm8[:nrow, :], -1.0,
                            sm8[:nrow, :], ALU.mult, ALU.mult)
                        nc.vector.scalar_tensor_tensor(
                            var8[:nrow, :], psq8[:nrow, :], 1.0,
                            var8[:nrow, :], ALU.mult, ALU.add)
                        nc.scalar.activation(sd8[:nrow, :], var8[:nrow, :],
                                             AF.Sqrt, bias=epsc[:nrow, :],
                                             scale=1.0)
                        nc.vector.reciprocal_approx_fast(rstd8[:nrow, :],
                                                         sd8[:nrow, :])
                        nc.scalar.copy(rstd8_bf[:nrow, :], rstd8[:nrow, :])
                        nc.vector.scalar_tensor_tensor(
                            affr8[:nrow, :], sm8[:nrow, :], -1.0,
                            rstd8[:nrow, :], ALU.mult, ALU.mult)
                        for jl, pr in enumerate(prs):
                            h1 = h1s[pr]
                            pscale = pp1.tile([128, GROUP], f32,
                                              tag="pscale", bufs=2)
                            poff = pp1.tile([128, GROUP], f32, tag="pscale",
                                            bufs=2, name="poff")
                            sl = slice(32 * jl, 32 * jl + 2)
                            nc.tensor.matmul(
                                pscale[:], ga_s[sl, :], rstd8_bf[sl, :])
                            nc.tensor.matmul(poff[:], ga_s[sl, :],
                                             affr8[sl, :])
                            t1t = pm.tile([128, GROUP], f32, tag="t1t",
                                          bufs=3)
                            h3 = pm.tile([128, GROUP], bf16, tag="h3",
                                         bufs=3)
                            nc.vector.tensor_mul(t1t[:], h1[:], pscale[:])
                            nc.vector.tensor_add(t1t[:], t1t[:], poff[:])
                            nc.scalar.activation(h3[:], t1t[:], AF.Relu,
                                                 bias=beta_c[:], scale=1.0)
                            pL2 = pp1.tile([2, GROUP], f32, tag="pscale", bufs=2,
                                           name="pL2")
                            nc.tensor.matmul(pL2[:], w2_s[:], h3[:])
                            lgs = pm.tile([2, GROUP], f32, tag="lgs",
                                          bufs=3)
                            nc.vector.tensor_copy(lgs[:], pL2[:])
                            nc.sync.dma_start(
                                bass.AP(out, t0 + pr * 2 * GROUP,
                                        [[GROUP, 2], [1, GROUP]]),
                                lgs[:])
                    nb0 += nbs

    nc.compile()
    return nc


def build_in_maps(cfg, inputs, preA, preB, wts):
    import ml_dtypes
    bf = ml_dtypes.bfloat16
    emb_full = np.asarray(inputs["embedding"], dtype=np.float32)
    B, N, NS, NBLK, BH, TA = (cfg["B"], cfg["N"], cfg["NS"], cfg["NBLK"],
                              cfg["BH"], cfg["TA"])
    # [N, B*H] view of the embedding
    embT = np.ascontiguousarray(emb_full.transpose(1, 0, 2).reshape(N, BH))
    in_maps = []
    for g in range(cfg["n_cores"]):
        dA, dB = preA[g], preB[g]
        embA_a = np.zeros((TA, BH), dtype=bf)
        valid = dA["rowsrcA"] >= 0
        src_nodes = dA["mnode"][dA["rowsrcA"][valid]]
        embA_a[valid] = (embT[src_nodes] *
                         dA["minvc"][dA["rowsrcA"][valid]][:, None]).astype(bf)
        embA_a = np.ascontiguousarray(
            embA_a.reshape(-1, 128, BH).transpose(1, 0, 2))
        embN_a = np.zeros((NS, BH), dtype=bf)
        n0 = g * NS
        n1 = min(n0 + NS, N)
        deg = preB[g]["deg"][:n1 - n0]
        embN_a[:n1 - n0] = (embT[n0:n1] * deg[:, None]).astype(bf)
        embN_a = np.ascontiguousarray(
            embN_a.reshape(-1, 128, BH).transpose(1, 0, 2))
        m = dict(embA=embA_a, indA=dA["indA"],
                 embN=embN_a, gidxB=dB["gidxB"], indwB=dB["indwB"],
                 invn_tok=dB["invn_tok"], **wts)
        in_maps.append(m)
    return in_maps


def assemble_out(cfg, results, b2v=0.0):
    B, N, NS, NBLK, GROUP = (cfg["B"], cfg["N"], cfg["NS"], cfg["NBLK"],
                             cfg["GROUP"])
    out = np.empty((B, N), dtype=np.float32)
    for g in range(cfg["n_cores"]):
        a = np.asarray(results[g]["out"]).reshape(-1, 2, 4, 128) + b2v
        toks = a.transpose(0, 2, 1, 3).reshape(-1)   # token-major
        # token t = (nb*B + b)*128 + p ; node = g*NS + nb*128 + p
        t = toks.reshape(NBLK, B, 128)
        n0 = g * NS
        n1 = min(n0 + NS, N)
        for b in range(B):
            out[b, n0:n1] = t[:, b, :].reshape(-1)[:n1 - n0]
    return out


def kernel(**inputs):
    emb = np.asarray(inputs["embedding"])
    B, N, _ = emb.shape
    C = int(inputs["num_classes"])
    E = len(np.asarray(inputs["n2c_row"]))
    cfg = make_cfg(B, N, C, E)
    preA, preB, meta = host_prep(cfg, inputs)
    wts = weight_tensors(inputs)
    wvals = dict(b2=float(np.asarray(inputs["b2"]).reshape(-1)[0]))
    nc = build(cfg, meta, wvals)
    in_maps = build_in_maps(cfg, inputs, preA, preB, wts)
    from concourse.bass_utils import run_bass_kernel_spmd
    res = run_bass_kernel_spmd(nc, in_maps,
                               core_ids=list(range(cfg["n_cores"])))
    return assemble_out(cfg, res.results, wvals["b2"])


# revision 4
# speedup vs baseline: 1.0716x; 1.0103x over previous
"""V4: class-sliced stage A + node-partitioned stage B, batched table rows.

Core g (of 8) owns:
  - stage A: class slices [g*CSH,(g+1)*CSH) in BOTH table halves; computes
    class-mean rows [class, B*H] via indicator matmuls from host-presorted
    member streams; two 8-way AllGathers assemble tableL/tableH (each
    [CPAD/2, B*H] bf16, < 32768 rows so int16 gather indices work).
  - stage B: nodes [g*NS,(g+1)*NS) for ALL batches; per edge one dma_gather
    of a 512B row (all 4 batches), indicator-weight matmuls (invn folded into
    host indw) accumulate ctx per 128-node block; emb added via identity
    matmul; fused LayerNorm MLP identical to the V3 scheme.
Low-half gathers overlap the second AllGather.
"""

import numpy as np

H = 64
LN_EPS = 1e-5
TG = 1024


def _ru(x, m):
    return (x + m - 1) // m * m


def _wrap16(idx):
    n = len(idx)
    n16 = _ru(n, 16)
    a = np.full(n16, -1, dtype=np.int16)
    a[:n] = idx
    a = a.reshape(n16 // 16, 16).T
    return np.tile(a, (8, 1)).copy()


def make_cfg(B, N, C, E):
    assert B == 4
    cfg = dict(B=B, N=N, C=C, E=E, n_cores=8)
    cfg["NPAD"] = _ru(N, 8 * 128)
    cfg["NS"] = cfg["NPAD"] // 8          # nodes per core
    cfg["NBLK"] = cfg["NS"] // 128
    cfg["CPAD"] = _ru(C, 16 * 128)
    cfg["SPLIT"] = min(32768, cfg["CPAD"] - cfg["CPAD"] // 2)
    assert cfg["SPLIT"] % 1024 == 0
    cfg["CS"] = [cfg["SPLIT"] // 8, (cfg["CPAD"] - cfg["SPLIT"]) // 8]
    cfg["CB"] = [cs // 128 for cs in cfg["CS"]]
    assert cfg["SPLIT"] <= 32768 and cfg["CPAD"] - cfg["SPLIT"] <= 32768
    cfg["GROUP"] = 512
    cfg["BH"] = B * H                     # 256
    return cfg


def host_prep(cfg, inputs):
    B, N, CPAD = cfg["B"], cfg["N"], cfg["CPAD"]
    CS, CB, SPLIT = cfg["CS"], cfg["CB"], cfg["SPLIT"]
    NS, NBLK = cfg["NS"], cfg["NBLK"]
    n_cores = cfg["n_cores"]
    c2n_row = np.asarray(inputs["c2n_row"]).astype(np.int64)
    c2n_col = np.asarray(inputs["c2n_col"]).astype(np.int64)
    n2c_row = np.asarray(inputs["n2c_row"]).astype(np.int64)
    n2c_col = np.asarray(inputs["n2c_col"]).astype(np.int64)

    cnt_c = np.bincount(c2n_row, minlength=CPAD).astype(np.float32)
    invc = (1.0 / np.maximum(cnt_c, 1.0)).astype(np.float32)
    cnt_n = np.bincount(n2c_row, minlength=cfg["NPAD"]).astype(np.float32)
    invn = (1.0 / np.maximum(cnt_n, 1.0)).astype(np.float32)

    # ---------------- stage A: member streams per (core, half) ----------
    # membership edges: node c2n_col[i] contributes to class c2n_row[i]
    order = np.argsort(c2n_row, kind="stable")
    mcls = c2n_row[order]           # sorted classes
    mnode = c2n_col[order]
    # core/half/block of each member
    mhalf = (mcls >= SPLIT).astype(np.int64)
    mloc = mcls - mhalf * SPLIT     # class index within half
    csz = np.where(mhalf == 1, CS[1], CS[0])
    mcore = mloc // csz
    mblk = (mloc % csz) // 128      # local block within (core, half)
    capsA = [None, None]
    for h in (0, 1):
        cnt = np.zeros((n_cores, CB[h]), dtype=np.int64)
        mh = mhalf == h
        np.add.at(cnt, (mcore[mh], mblk[mh]), 1)
        capsA[h] = _ru(np.maximum(cnt.max(axis=0), 1), 128)
    TAH = [_ru(int(capsA[h].sum()), TG) for h in (0, 1)]
    cfg["TAH"] = TAH
    cfg["TA"] = TAH[0] + TAH[1]
    schedA = []                       # per 128-chunk: (half, local block) or (h,-1)
    for h in (0, 1):
        nchunk = 0
        for j in range(CB[h]):
            for _ in range(capsA[h][j] // 128):
                schedA.append((h, j))
                nchunk += 1
        for _ in range(TAH[h] // 128 - nchunk):
            schedA.append((h, -1))

    # per-core stage A padded slots
    preA = []
    for g in range(n_cores):
        rowsrc = np.full(cfg["TA"], -1, dtype=np.int64)
        segA = np.full(cfg["TA"], 255, dtype=np.float32)
        base = 0
        for h in (0, 1):
            for j in range(CB[h]):
                m = (mcore == g) & (mhalf == h) & (mblk == j)
                nm = int(m.sum())
                rowsrc[base:base + nm] = np.nonzero(m)[0]
                segA[base:base + nm] = (mloc[m] % CS[h]) % 128
                base += capsA[h][j]
            base = TAH[0]
        import ml_dtypes
        bf = ml_dtypes.bfloat16
        import ml_dtypes as _md
        f8a = _md.float8_e4m3fn
        ia = np.zeros((cfg["TA"], 128), dtype=np.float32)
        vv = segA < 255
        ia[np.nonzero(vv)[0], segA[vv].astype(np.int64)] = 1.0
        indA_w = np.ascontiguousarray(
            ia.astype(f8a).reshape(-1, 128, 128).transpose(1, 0, 2))
        preA.append(dict(rowsrcA=rowsrc, indA=indA_w,
                         mnode=mnode, minvc=invc[mcls]))

    # ---------------- stage B: edge streams per core ---------------------
    ecore = n2c_row // NS
    ehalf = (n2c_col >= SPLIT).astype(np.int64)
    erow = n2c_col - ehalf * SPLIT    # gather row within half table
    edstl = n2c_row - ecore * NS      # local dst node
    enb = edstl // 128
    cntB = np.zeros((n_cores, 2, NBLK), dtype=np.int64)
    np.add.at(cntB, (ecore, ehalf, enb), 1)
    capsB = _ru(np.maximum(cntB.max(axis=0), 1), 128)   # [2, NBLK]
    TBH_ = [int(capsB[h].sum()) for h in (0, 1)]
    TBH_ = [_ru(t, TG) for t in TBH_]
    cfg["TBL"], cfg["TBHI"] = TBH_[0], TBH_[1]
    cfg["TB"] = TBH_[0] + TBH_[1]
    schedB = []
    for h in (0, 1):
        nchunk = 0
        for nb in range(NBLK):
            for _ in range(capsB[h, nb] // 128):
                schedB.append((h, nb))
                nchunk += 1
        for _ in range(TBH_[h] // 128 - nchunk):
            schedB.append((h, -1))

    preB = []
    import ml_dtypes
    f8 = ml_dtypes.float8_e4m3fn
    for g in range(n_cores):
        gidx = ((np.arange(cfg["TB"], dtype=np.int64) * 97) % 64)
        iw = np.zeros((cfg["TB"], 128), dtype=np.float32)
        base = 0
        for h in (0, 1):
            for nb in range(NBLK):
                m = (ecore == g) & (ehalf == h) & (enb == nb)
                nm = int(m.sum())
                gidx[base:base + nm] = erow[m]
                iw[np.arange(base, base + nm), edstl[m] % 128] = 1.0
                base += capsB[h, nb]
            base = TBH_[0]
        iw_w = np.ascontiguousarray(
            iw.astype(f8).reshape(-1, 128, 128).transpose(1, 0, 2))
        ivn = np.zeros((128, NBLK), dtype=np.float32)
        n0 = g * NS
        ivn[:, :] = invn[n0:n0 + NS].reshape(NBLK, 128).T
        deg = np.maximum(cnt_n[n0:n0 + NS], 1.0)
        preB.append(dict(gidxB=_wrap16(gidx), indwB=iw_w, invn_tok=ivn,
                         deg=deg))

    meta = dict(schedA=schedA, schedB=schedB)
    return preA, preB, meta


def weight_tensors(inputs):
    import ml_dtypes
    bf = ml_dtypes.bfloat16
    W1 = np.asarray(inputs["W1"], dtype=np.float32)
    b1 = np.asarray(inputs["b1"], dtype=np.float32)
    gamma = np.asarray(inputs["gamma"], dtype=np.float32)
    beta = np.asarray(inputs["beta"], dtype=np.float32)
    W2 = np.asarray(inputs["W2"], dtype=np.float32)
    w1blk = np.zeros((128, 128), dtype=np.float32)
    w1blk[:H, :H] = W1
    w1blk[H:, H:] = W1
    b1col = np.concatenate([b1, b1]).reshape(128, 1).astype(np.float32)
    stats = np.zeros((128, 2), dtype=np.float32)
    stats[:H, 0] = 1.0 / H
    stats[H:, 1] = 1.0 / H
    gamma2 = np.zeros((128, 128), dtype=np.float32)
    beta2 = np.zeros((128, 128), dtype=np.float32)
    for base in (0, 32, 64):
        gamma2[base, :H] = gamma
        gamma2[base + 1, H:] = gamma
        beta2[base, :H] = beta
        beta2[base + 1, H:] = beta
    w2col = np.zeros((128, 2), dtype=np.float32)
    w2col[:H, 0] = W2[:, 0]
    w2col[H:, 1] = W2[:, 0]
    iota = np.tile(np.arange(128, dtype=np.float32), (128, 1))
    beta2col = np.concatenate([beta, beta]).reshape(128, 1).astype(np.float32)
    return dict(
        w1blk=w1blk.astype(bf), b1col=b1col, beta2col=beta2col,
        stats_lhsT=stats.astype(bf), gamma2=gamma2.astype(bf),
        beta2=beta2.astype(bf), w2col=w2col.astype(bf),
        identd=np.eye(128, dtype=np.float32).astype(bf),
        iota_d=iota.astype(bf))


def build(cfg, meta, wvals):
    from concourse import bass, bacc, tile, mybir

    f32 = mybir.dt.float32
    bf16 = mybir.dt.bfloat16
    fp8 = mybir.dt.float8e4
    i16 = mybir.dt.int16
    AF = mybir.ActivationFunctionType
    ALU = mybir.AluOpType

    B, BH = cfg["B"], cfg["BH"]
    NBLK, CS, CB, SPLIT = cfg["NBLK"], cfg["CS"], cfg["CB"], cfg["SPLIT"]
    CPAD = cfg["CPAD"]
    TA, TAH = cfg["TA"], cfg["TAH"]
    TB, TBL = cfg["TB"], cfg["TBL"]
    GROUP = cfg["GROUP"]
    n_cores = cfg["n_cores"]
    schedA, schedB = meta["schedA"], meta["schedB"]
    KC = TG // 128                       # 8 chunks per gather call
    NTOKC = NBLK * B * 128               # tokens per core
    assert NTOKC % GROUP == 0

    nc = bacc.Bacc("TRN2", target_bir_lowering=False, debug=False,
                   num_devices=n_cores, num_swdge_queues=4)

    embA = nc.dram_tensor("embA", [128, TA // 128, BH], bf16,
                          kind="ExternalInput")
    indA = nc.dram_tensor("indA", [128, TA // 128, 128], fp8,
                          kind="ExternalInput")
    embN = nc.dram_tensor("embN", [128, NBLK, BH], bf16, kind="ExternalInput")
    gidxB = nc.dram_tensor("gidxB", [128, TB // 16], i16, kind="ExternalInput")
    indwB = nc.dram_tensor("indwB", [128, TB // 128, 128], fp8,
                           kind="ExternalInput")
    iota_d = nc.dram_tensor("iota_d", [128, 128], bf16, kind="ExternalInput")
    w1blk = nc.dram_tensor("w1blk", [128, 128], bf16, kind="ExternalInput")
    b1col = nc.dram_tensor("b1col", [128, 1], f32, kind="ExternalInput")
    stats_lhsT = nc.dram_tensor("stats_lhsT", [128, 2], bf16,
                                kind="ExternalInput")
    gamma2 = nc.dram_tensor("gamma2", [128, 128], bf16, kind="ExternalInput")
    beta2 = nc.dram_tensor("beta2", [128, 128], bf16, kind="ExternalInput")
    w2col = nc.dram_tensor("w2col", [128, 2], bf16, kind="ExternalInput")
    identd = nc.dram_tensor("identd", [128, 128], bf16, kind="ExternalInput")
    beta2col = nc.dram_tensor("beta2col", [128, 1], f32,
                              kind="ExternalInput")
    out = nc.dram_tensor("out", [NTOKC // GROUP, GROUP], f32,
                         kind="ExternalOutput")

    chslice = nc.dram_tensor("chslice", [CS[0] + CS[1], BH], fp8)
    tableL = nc.dram_tensor("tableL", [SPLIT, BH], fp8,
                            addr_space="Shared")
    tableH = nc.dram_tensor("tableH", [CPAD - SPLIT, BH], fp8,
                            addr_space="Shared")
    invn_tok = nc.dram_tensor("invn_tok", [128, NBLK], f32,
                              kind="ExternalInput")

    b2v = float(wvals["b2"])
    groups8 = [list(range(n_cores))]

    with tile.TileContext(nc) as tc:
        AKC = 2 * KC  # embA DMA chunk: 2048 rows
        with tc.tile_pool(name="pw", bufs=1) as pw:
            w1_s = pw.tile([128, 128], bf16)
            b1_s = pw.tile([128, 1], f32)
            st_s = pw.tile([128, 2], bf16)
            ga_s = pw.tile([128, 128], bf16)
            be_s = pw.tile([128, 128], bf16)
            w2_s = pw.tile([128, 2], bf16)
            ident = pw.tile([128, 128], bf16)
            ones2 = pw.tile([128, GROUP], bf16)
            epsc = pw.tile([128, 1], f32)
            beta_c = pw.tile([128, 1], f32)
            ib_t = pw.tile([128, TB // 16], i16)
            iota_s = pw.tile([128, 128], bf16)
            ivn_s = pw.tile([128, NBLK], f32)
            nc.scalar.dma_start(w1_s[:], w1blk[:, :])
            nc.scalar.dma_start(b1_s[:], b1col[:, :])
            nc.scalar.dma_start(st_s[:], stats_lhsT[:, :])
            nc.scalar.dma_start(ga_s[:], gamma2[:, :])
            nc.scalar.dma_start(be_s[:], beta2[:, :])
            nc.scalar.dma_start(w2_s[:], w2col[:, :])
            nc.scalar.dma_start(ident[:], identd[:, :])
            nc.scalar.dma_start(ib_t[:], gidxB[:, :])
            nc.scalar.dma_start(iota_s[:], iota_d[:, :])
            nc.scalar.dma_start(ivn_s[:], invn_tok[:, :])
            nc.vector.memset(ones2[:], 1.0)
            nc.vector.memset(epsc[:], LN_EPS)
            nc.scalar.dma_start(beta_c[:], beta2col[:, :])

            # ================= stage A (per half) =================
            ck_base = [0, TAH[0] // 128]
            for h2 in (0, 1):
                with tc.tile_pool(name=f"pAg{h2}", bufs=4) as pAg, \
                     tc.tile_pool(name=f"pAn{h2}", bufs=1) as pAn, \
                     tc.tile_pool(name=f"pAp{h2}", bufs=2,
                                  space="PSUM") as pAp:
                    sbt = pAn.tile([128, CB[h2], BH], fp8, tag="sbt")
                    ck_lo = ck_base[h2]
                    ck_hi = ck_lo + TAH[h2] // 128
                    state = dict(tile=None, ind=None, pend=[], prev=None)

                    def flushA():
                        pend = state["pend"]
                        if not pend:
                            return
                        j = state["prev"]
                        ps = pAp.tile([128, BH], f32, tag="psA")
                        for i, (tl, ind_t, kk) in enumerate(pend):
                            nc.tensor.matmul(ps[:], ind_t[:, kk, :],
                                             tl[:, kk, :], start=(i == 0),
                                             stop=(i == len(pend) - 1))
                        nc.scalar.copy(sbt[:, j:j + 1, :],
                                       ps[:].unsqueeze(1))

                    for ck in range(ck_lo, ck_hi):
                        _h, j = schedA[ck]
                        if (ck - ck_lo) % AKC == 0:
                            n = min(AKC, ck_hi - ck)
                            state["tile"] = pAg.tile([128, AKC, BH], bf16,
                                                     tag="gA", name="gA")
                            eng = [nc.scalar, nc.sync][(ck // AKC) % 2]
                            eng.dma_start(
                                state["tile"][:, :n, :],
                                bass.AP(embA, ck * BH,
                                        [[TA // 128 * BH, 128], [BH, n],
                                         [1, BH]]))
                            state["ind"] = pAg.tile([128, AKC, 128], fp8,
                                                    tag="indA", name="indAt")
                            eng2 = [nc.sync, nc.scalar][(ck // AKC) % 2]
                            eng2.dma_start(
                                state["ind"][:, :n, :],
                                bass.AP(indA, ck * 128,
                                        [[TA // 128 * 128, 128], [128, n],
                                         [1, 128]]))
                        if j < 0:
                            continue
                        if j != state["prev"]:
                            flushA()
                            state["pend"] = []
                            state["prev"] = j
                        state["pend"].append(
                            (state["tile"], state["ind"], (ck - ck_lo) % AKC))
                    flushA()
                    cbase = 0 if h2 == 0 else CS[0]
                    for j0 in range(0, CB[h2], 5):
                        j1 = min(j0 + 5, CB[h2])
                        nc.sync.dma_start(
                            bass.AP(chslice, (cbase + j0 * 128) * BH,
                                    [[BH, 128], [128 * BH, j1 - j0],
                                     [1, BH]]),
                            sbt[:, j0:j1, :])
                tab = tableL if h2 == 0 else tableH
                trows = SPLIT if h2 == 0 else CPAD - SPLIT
                nc.gpsimd.collective_compute(
                    "AllGather", mybir.AluOpType.bypass,
                    replica_groups=groups8,
                    ins=[bass.AP(chslice, (0 if h2 == 0 else CS[0]) * BH,
                                 [[1, CS[h2] * BH]]).opt()],
                    outs=[bass.AP(tab, 0, [[1, trows * BH]]).opt()])

            # ================= stage B =================
            capL = bass.AP(tableL, 0, [[BH, SPLIT], [1, BH]])
            capH = bass.AP(tableH, 0, [[BH, CPAD - SPLIT], [1, BH]])

            # chunk lists per (half, nb)
            blk_chunks = {0: {}, 1: {}}
            for ck, (h, nb) in enumerate(schedB):
                if nb >= 0:
                    blk_chunks[h].setdefault(nb, []).append(ck)

            gtiles = {}

            with tc.tile_pool(name="pm", bufs=2) as pm, \
                 tc.tile_pool(name="pBg", bufs=8) as pBg, \
                 tc.tile_pool(name="pctx", bufs=1) as pctx, \
                 tc.tile_pool(name="pp", bufs=2, space="PSUM") as pp, \
                 tc.tile_pool(name="pp1", bufs=1, space="PSUM") as pp1, \
                 tc.tile_pool(name="ppB", bufs=2, space="PSUM") as ppB:

                ctxL = pctx.tile([128, NBLK, BH], bf16)

                def get_gtile(call):
                    if call not in gtiles:
                        src = capL if call < TBL // TG else capH
                        t = pBg.tile([128, KC, BH], fp8, tag="gB", bufs=10,
                                     name="gB")
                        nc.gpsimd.dma_gather(
                            t[:], src,
                            ib_t[:, call * TG // 16:(call + 1) * TG // 16],
                            TG, TG, BH, queue_num=call % 4)
                        iw = pBg.tile([128, KC, 128], fp8, tag="iwB", bufs=10,
                                      name="iwB")
                        nc.sync.dma_start(
                            iw[:],
                            bass.AP(indwB, call * KC * 128,
                                    [[TB, 128], [128, KC], [1, 128]]))
                        gtiles[call] = (t, iw)
                    return gtiles[call]

                # ---- LOW pass: ctxL per block (incl. emb*deg via ident)
                lastL = {nb: max(cks) // KC
                         for nb, cks in blk_chunks[0].items()}
                ncall_L = TBL // TG
                nbdone = 0
                etL = {}
                NES = 32
                for cg in range(0, ncall_L, 8):
                    for call in range(cg, min(cg + 8, ncall_L)):
                        get_gtile(call)
                    cov = min(cg + 8, ncall_L) - 1
                    while nbdone < NBLK and lastL.get(nbdone, -1) <= cov:
                        nb = nbdone
                        es = nb // NES
                        if es not in etL:
                            ne = min(NES, NBLK - es * NES)
                            etl = pm.tile([128, NES, BH], bf16, tag="embL",
                                          bufs=2, name="etl")
                            nc.scalar.dma_start(
                                etl[:, :ne, :],
                                bass.AP(embN, es * NES * BH,
                                        [[NBLK * BH, 128], [BH, ne],
                                         [1, BH]]))
                            etL[es] = etl
                        cks = blk_chunks[0].get(nb, [])
                        ps = ppB.tile([128, BH], f32, tag="psB")
                        nc.tensor.matmul(ps[:], ident[:],
                                         etL[es][:, nb % NES, :],
                                         start=True, stop=False)
                        for i, ck in enumerate(cks):
                            tl, iw = get_gtile(ck // KC)
                            nc.tensor.matmul(ps[:], iw[:, ck % KC, :],
                                             tl[:, ck % KC, :],
                                             start=False,
                                             stop=(i == len(cks) - 1))
                        nc.scalar.copy(ctxL[:, nb:nb + 1, :],
                                       ps[:].unsqueeze(1))
                        nbdone += 1

                # ---- HIGH pass + MLP, slabs of SLABB blocks
                SLABB = 24
                lastH = {nb: (max(cks) - TBL // 128) // KC
                         for nb, cks in blk_chunks[1].items()}
                ncall_H = (TB - TBL) // TG
                gpc = GROUP // 128
                nb0 = 0
                issuedH = -1
                while nb0 < NBLK:
                    nbs = min(SLABB, NBLK - nb0)
                    t0 = nb0 * B * 128           # token base
                    xt = pm.tile([128, SLABB, BH], bf16, tag="xm", bufs=3)
                    jdone = 0
                    while jdone < nbs:
                        if issuedH < ncall_H - 1:
                            hi = min(issuedH + 8, ncall_H - 1)
                            for call in range(issuedH + 1, hi + 1):
                                get_gtile(ncall_L + call)
                            issuedH = hi
                        while jdone < nbs and \
                                lastH.get(nb0 + jdone, -1) <= issuedH:
                            j = jdone
                            nb = nb0 + j
                            cks = blk_chunks[1].get(nb, [])
                            ps = ppB.tile([128, BH], f32, tag="psB")
                            nc.tensor.matmul(ps[:], ident[:],
                                             ctxL[:, nb, :],
                                             start=True, stop=False)
                            for i, ck in enumerate(cks):
                                call = ncall_L + (ck - TBL // 128) // KC
                                tl, iw = get_gtile(call)
                                kk = (ck - TBL // 128) % KC
                                nc.tensor.matmul(ps[:], iw[:, kk, :],
                                                 tl[:, kk, :],
                                                 start=False,
                                                 stop=(i == len(cks) - 1))
                            nc.vector.tensor_scalar(
                                xt[:, j:j + 1, :], ps[:].unsqueeze(1),
                                ivn_s[:, nb:nb + 1], None,
                                ALU.mult, ALU.bypass)
                            jdone += 1

                    # ---- MLP on this slab
                    ntok = nbs * B * 128
                    npr = ntok // (2 * GROUP)
                    for pg0 in range(0, npr, 3):
                        prs = list(range(pg0, min(pg0 + 3, npr)))
                        pst8 = pp1.tile([128, GROUP], f32, tag="pst8")
                        psq8 = pp1.tile([128, GROUP], f32, tag="psq8")
                        h1s = {}
                        for jl, pr in enumerate(prs):
                            xT = pp.tile([128, GROUP], bf16, tag="mmp")
                            for jj in range(gpc):
                                c0 = 2 * (pr * gpc + jj)
                                nc.tensor.transpose(
                                    xT[:, jj * 128:(jj + 1) * 128],
                                    xt[:, c0 // B,
                                       (c0 % B) * H:(c0 % B) * H + 2 * H]
                                    .unsqueeze(1), ident[:])
                            xT_sb = pm.tile([128, GROUP], bf16, tag="xTsb",
                                            bufs=3)
                            nc.scalar.copy(xT_sb[:], xT[:])
                            ph = pp.tile([128, GROUP], f32, tag="mmp")
                            nc.tensor.matmul(ph[:], w1_s[:], xT_sb[:])
                            h1 = pm.tile([128, GROUP], bf16, tag="h1",
                                         bufs=6)
                            sq = pm.tile([128, GROUP], bf16, tag="sq",
                                         bufs=3)
                            nc.scalar.activation(h1[:], ph[:], AF.Identity,
                                                 bias=b1_s[:], scale=1.0)
                            nc.scalar.activation(sq[:], h1[:], AF.Square)
                            nc.tensor.matmul(
                                pst8[32 * jl:32 * jl + 2, :], st_s[:], h1[:])
                            nc.tensor.matmul(
                                psq8[32 * jl:32 * jl + 2, :], st_s[:], sq[:])
                            h1s[pr] = h1
                        nrow = 32 * (len(prs) - 1) + 2
                        sm8 = pm.tile([128, GROUP], f32, tag="sm8", bufs=2)
                        var8 = pm.tile([128, GROUP], f32, tag="var8", bufs=2)
                        sd8 = pm.tile([128, GROUP], f32, tag="sd8", bufs=2)
                        rstd8 = pm.tile([128, GROUP], f32, tag="rstd8",
                                        bufs=2)
                        rstd8_bf = pm.tile([128, GROUP], bf16, tag="rstd8b",
                                           bufs=2)
                        affr8 = pm.tile([128, GROUP], bf16, tag="affr8",
                                        bufs=2)
                        nc.scalar.copy(sm8[:nrow, :], pst8[:nrow, :])
                        nc.vector.scalar_tensor_tensor(
                            var8[:nrow, :], sm8[:nrow, :], -1.0,
                            sm8[:nrow, :], ALU.mult, ALU.mult)
                        nc.vector.scalar_tensor_tensor(
                            var8[:nrow, :], psq8[:nrow, :], 1.0,
                            var8[:nrow, :], ALU.mult, ALU.add)
                        nc.scalar.activation(sd8[:nrow, :], var8[:nrow, :],
                                             AF.Sqrt, bias=epsc[:nrow, :],
                                             scale=1.0)
                        nc.vector.reciprocal_approx_fast(rstd8[:nrow, :],
                                                         sd8[:nrow, :])
                        nc.scalar.copy(rstd8_bf[:nrow, :], rstd8[:nrow, :])
                        nc.vector.scalar_tensor_tensor(
                            affr8[:nrow, :], sm8[:nrow, :], -1.0,
                            rstd8[:nrow, :], ALU.mult, ALU.mult)
                        for jl, pr in enumerate(prs):
                            h1 = h1s[pr]
                            pscale = pp1.tile([128, GROUP], f32,
                                              tag="pscale", bufs=2)
                            poff = pp1.tile([128, GROUP], f32, tag="pscale",
                                            bufs=2, name="poff")
                            sl = slice(32 * jl, 32 * jl + 2)
                            nc.tensor.matmul(
                                pscale[:], ga_s[sl, :], rstd8_bf[sl, :])
                            nc.tensor.matmul(poff[:], ga_s[sl, :],
                                             affr8[sl, :])
                            t1t = pm.tile([128, GROUP], f32, tag="t1t",
                                          bufs=3)
                            h3 = pm.tile([128, GROUP], bf16, tag="h3",
                                         bufs=3)
                            nc.vector.tensor_mul(t1t[:], h1[:], pscale[:])
                            nc.vector.tensor_add(t1t[:], t1t[:], poff[:])
                            nc.scalar.activation(h3[:], t1t[:], AF.Relu,
                                                 bias=beta_c[:], scale=1.0)
                            pL2 = pp1.tile([2, GROUP], f32, tag="pscale", bufs=2,
                                           name="pL2")
                            nc.tensor.matmul(pL2[:], w2_s[:], h3[:])
                            lgs = pm.tile([2, GROUP], f32, tag="lgs",
                                          bufs=3)
                            nc.vector.tensor_copy(lgs[:], pL2[:])
                            nc.sync.dma_start(
                                bass.AP(out, t0 + pr * 2 * GROUP,
                                        [[GROUP, 2], [1, GROUP]]),
                                lgs[:])
                    nb0 += nbs

    nc.compile()
    return nc


def build_in_maps(cfg, inputs, preA, preB, wts):
    import ml_dtypes
    bf = ml_dtypes.bfloat16
    emb_full = np.asarray(inputs["embedding"], dtype=np.float32)
    B, N, NS, NBLK, BH, TA = (cfg["B"], cfg["N"], cfg["NS"], cfg["NBLK"],
                              cfg["BH"], cfg["TA"])
    # [N, B*H] view of the embedding
    embT = np.ascontiguousarray(emb_full.transpose(1, 0, 2).reshape(N, BH))
    in_maps = []
    for g in range(cfg["n_cores"]):
        dA, dB = preA[g], preB[g]
        embA_a = np.zeros((TA, BH), dtype=bf)
        valid = dA["rowsrcA"] >= 0
        src_nodes = dA["mnode"][dA["rowsrcA"][valid]]
        embA_a[valid] = (embT[src_nodes] *
                         dA["minvc"][dA["rowsrcA"][valid]][:, None]).astype(bf)
        embA_a = np.ascontiguousarray(
            embA_a.reshape(-1, 128, BH).transpose(1, 0, 2))
        embN_a = np.zeros((NS, BH), dtype=bf)
        n0 = g * NS
        n1 = min(n0 + NS, N)
        deg = preB[g]["deg"][:n1 - n0]
        embN_a[:n1 - n0] = (embT[n0:n1] * deg[:, None]).astype(bf)
        embN_a = np.ascontiguousarray(
            embN_a.reshape(-1, 128, BH).transpose(1, 0, 2))
        m = dict(embA=embA_a, indA=dA["indA"],
                 embN=embN_a, gidxB=dB["gidxB"], indwB=dB["indwB"],
                 invn_tok=dB["invn_tok"], **wts)
        in_maps.append(m)
    return in_maps


def assemble_out(cfg, results, b2v=0.0):
    B, N, NS, NBLK, GROUP = (cfg["B"], cfg["N"], cfg["NS"], cfg["NBLK"],
                             cfg["GROUP"])
    out = np.empty((B, N), dtype=np.float32)
    for g in range(cfg["n_cores"]):
        a = np.asarray(results[g]["out"]).reshape(-1, 2, 4, 128) + b2v
        toks = a.transpose(0, 2, 1, 3).reshape(-1)   # token-major
        # token t = (nb*B + b)*128 + p ; node = g*NS + nb*128 + p
        t = toks.reshape(NBLK, B, 128)
        n0 = g * NS
        n1 = min(n0 + NS, N)
        for b in range(B):
            out[b, n0:n1] = t[:, b, :].reshape(-1)[:n1 - n0]
    return out


def kernel(**inputs):
    emb = np.asarray(inputs["embedding"])
    B, N, _ = emb.shape
    C = int(inputs["num_classes"])
    E = len(np.asarray(inputs["n2c_row"]))
    cfg = make_cfg(B, N, C, E)
    preA, preB, meta = host_prep(cfg, inputs)
    wts = weight_tensors(inputs)
    wvals = dict(b2=float(np.asarray(inputs["b2"]).reshape(-1)[0]))
    nc = build(cfg, meta, wvals)
    in_maps = build_in_maps(cfg, inputs, preA, preB, wts)
    from concourse.bass_utils import run_bass_kernel_spmd
    res = run_bass_kernel_spmd(nc, in_maps,
                               core_ids=list(range(cfg["n_cores"])))
    return assemble_out(cfg, res.results, wvals["b2"])


# revision 5
# speedup vs baseline: 1.0785x; 1.0064x over previous
"""V4: class-sliced stage A + node-partitioned stage B, batched table rows.

Core g (of 8) owns:
  - stage A: class slices [g*CSH,(g+1)*CSH) in BOTH table halves; computes
    class-mean rows [class, B*H] via indicator matmuls from host-presorted
    member streams; two 8-way AllGathers assemble tableL/tableH (each
    [CPAD/2, B*H] bf16, < 32768 rows so int16 gather indices work).
  - stage B: nodes [g*NS,(g+1)*NS) for ALL batches; per edge one dma_gather
    of a 512B row (all 4 batches), indicator-weight matmuls (invn folded into
    host indw) accumulate ctx per 128-node block; emb added via identity
    matmul; fused LayerNorm MLP identical to the V3 scheme.
Low-half gathers overlap the second AllGather.
"""

import numpy as np

H = 64
LN_EPS = 1e-5
TG = 1024


def _ru(x, m):
    return (x + m - 1) // m * m


def _wrap16(idx):
    n = len(idx)
    n16 = _ru(n, 16)
    a = np.full(n16, -1, dtype=np.int16)
    a[:n] = idx
    a = a.reshape(n16 // 16, 16).T
    return np.tile(a, (8, 1)).copy()


def make_cfg(B, N, C, E):
    assert B == 4
    cfg = dict(B=B, N=N, C=C, E=E, n_cores=8)
    cfg["NPAD"] = _ru(N, 8 * 128)
    cfg["NS"] = cfg["NPAD"] // 8          # nodes per core
    cfg["NBLK"] = cfg["NS"] // 128
    cfg["CPAD"] = _ru(C, 16 * 128)
    cfg["SPLIT"] = min(32768, cfg["CPAD"] - cfg["CPAD"] // 2)
    assert cfg["SPLIT"] % 1024 == 0
    cfg["CS"] = [cfg["SPLIT"] // 8, (cfg["CPAD"] - cfg["SPLIT"]) // 8]
    cfg["CB"] = [cs // 128 for cs in cfg["CS"]]
    assert cfg["SPLIT"] <= 32768 and cfg["CPAD"] - cfg["SPLIT"] <= 32768
    cfg["GROUP"] = 512
    cfg["BH"] = B * H                     # 256
    return cfg


def host_prep(cfg, inputs):
    B, N, CPAD = cfg["B"], cfg["N"], cfg["CPAD"]
    CS, CB, SPLIT = cfg["CS"], cfg["CB"], cfg["SPLIT"]
    NS, NBLK = cfg["NS"], cfg["NBLK"]
    n_cores = cfg["n_cores"]
    c2n_row = np.asarray(inputs["c2n_row"]).astype(np.int64)
    c2n_col = np.asarray(inputs["c2n_col"]).astype(np.int64)
    n2c_row = np.asarray(inputs["n2c_row"]).astype(np.int64)
    n2c_col = np.asarray(inputs["n2c_col"]).astype(np.int64)

    cnt_c = np.bincount(c2n_row, minlength=CPAD).astype(np.float32)
    invc = (1.0 / np.maximum(cnt_c, 1.0)).astype(np.float32)
    cnt_n = np.bincount(n2c_row, minlength=cfg["NPAD"]).astype(np.float32)
    invn = (1.0 / np.maximum(cnt_n, 1.0)).astype(np.float32)

    # ---------------- stage A: member streams per (core, half) ----------
    # membership edges: node c2n_col[i] contributes to class c2n_row[i]
    order = np.argsort(c2n_row, kind="stable")
    mcls = c2n_row[order]           # sorted classes
    mnode = c2n_col[order]
    # core/half/block of each member
    mhalf = (mcls >= SPLIT).astype(np.int64)
    mloc = mcls - mhalf * SPLIT     # class index within half
    csz = np.where(mhalf == 1, CS[1], CS[0])
    mcore = mloc // csz
    mblk = (mloc % csz) // 128      # local block within (core, half)
    capsA = [None, None]
    for h in (0, 1):
        cnt = np.zeros((n_cores, CB[h]), dtype=np.int64)
        mh = mhalf == h
        np.add.at(cnt, (mcore[mh], mblk[mh]), 1)
        capsA[h] = _ru(np.maximum(cnt.max(axis=0), 1), 128)
    TAH = [_ru(int(capsA[h].sum()), TG) for h in (0, 1)]
    cfg["TAH"] = TAH
    cfg["TA"] = TAH[0] + TAH[1]
    schedA = []                       # per 128-chunk: (half, local block) or (h,-1)
    for h in (0, 1):
        nchunk = 0
        for j in range(CB[h]):
            for _ in range(capsA[h][j] // 128):
                schedA.append((h, j))
                nchunk += 1
        for _ in range(TAH[h] // 128 - nchunk):
            schedA.append((h, -1))

    # per-core stage A padded slots
    preA = []
    for g in range(n_cores):
        rowsrc = np.full(cfg["TA"], -1, dtype=np.int64)
        segA = np.full(cfg["TA"], 255, dtype=np.float32)
        base = 0
        for h in (0, 1):
            for j in range(CB[h]):
                m = (mcore == g) & (mhalf == h) & (mblk == j)
                nm = int(m.sum())
                rowsrc[base:base + nm] = np.nonzero(m)[0]
                segA[base:base + nm] = (mloc[m] % CS[h]) % 128
                base += capsA[h][j]
            base = TAH[0]
        import ml_dtypes
        bf = ml_dtypes.bfloat16
        import ml_dtypes as _md
        f8a = _md.float8_e4m3fn
        ia = np.zeros((cfg["TA"], 128), dtype=np.float32)
        vv = segA < 255
        ia[np.nonzero(vv)[0], segA[vv].astype(np.int64)] = 1.0
        indA_w = np.ascontiguousarray(
            ia.astype(f8a).reshape(-1, 128, 128).transpose(1, 0, 2))
        preA.append(dict(rowsrcA=rowsrc, indA=indA_w,
                         mnode=mnode, minvc=invc[mcls]))

    # ---------------- stage B: edge streams per core ---------------------
    ecore = n2c_row // NS
    ehalf = (n2c_col >= SPLIT).astype(np.int64)
    erow = n2c_col - ehalf * SPLIT    # gather row within half table
    edstl = n2c_row - ecore * NS      # local dst node
    enb = edstl // 128
    cntB = np.zeros((n_cores, 2, NBLK), dtype=np.int64)
    np.add.at(cntB, (ecore, ehalf, enb), 1)
    capsB = _ru(np.maximum(cntB.max(axis=0), 1), 128)   # [2, NBLK]
    TBH_ = [int(capsB[h].sum()) for h in (0, 1)]
    TBH_ = [_ru(t, TG) for t in TBH_]
    cfg["TBL"], cfg["TBHI"] = TBH_[0], TBH_[1]
    cfg["TB"] = TBH_[0] + TBH_[1]
    schedB = []
    for h in (0, 1):
        nchunk = 0
        for nb in range(NBLK):
            for _ in range(capsB[h, nb] // 128):
                schedB.append((h, nb))
                nchunk += 1
        for _ in range(TBH_[h] // 128 - nchunk):
            schedB.append((h, -1))

    preB = []
    import ml_dtypes
    f8 = ml_dtypes.float8_e4m3fn
    for g in range(n_cores):
        gidx = ((np.arange(cfg["TB"], dtype=np.int64) * 97) % 64)
        iw = np.zeros((cfg["TB"], 128), dtype=np.float32)
        base = 0
        for h in (0, 1):
            for nb in range(NBLK):
                m = (ecore == g) & (ehalf == h) & (enb == nb)
                nm = int(m.sum())
                gidx[base:base + nm] = erow[m]
                iw[np.arange(base, base + nm), edstl[m] % 128] = 1.0
                base += capsB[h, nb]
            base = TBH_[0]
        iw_w = np.ascontiguousarray(
            iw.astype(f8).reshape(-1, 128, 128).transpose(1, 0, 2))
        ivn = np.zeros((128, NBLK), dtype=np.float32)
        n0 = g * NS
        ivn[:, :] = invn[n0:n0 + NS].reshape(NBLK, 128).T
        deg = np.maximum(cnt_n[n0:n0 + NS], 1.0)
        preB.append(dict(gidxB=_wrap16(gidx), indwB=iw_w, invn_tok=ivn,
                         deg=deg))

    meta = dict(schedA=schedA, schedB=schedB)
    return preA, preB, meta


def weight_tensors(inputs):
    import ml_dtypes
    bf = ml_dtypes.bfloat16
    W1 = np.asarray(inputs["W1"], dtype=np.float32)
    b1 = np.asarray(inputs["b1"], dtype=np.float32)
    gamma = np.asarray(inputs["gamma"], dtype=np.float32)
    beta = np.asarray(inputs["beta"], dtype=np.float32)
    W2 = np.asarray(inputs["W2"], dtype=np.float32)
    w1blk = np.zeros((128, 128), dtype=np.float32)
    w1blk[:H, :H] = W1
    w1blk[H:, H:] = W1
    b1col = np.concatenate([b1, b1]).reshape(128, 1).astype(np.float32)
    stats = np.zeros((128, 2), dtype=np.float32)
    stats[:H, 0] = 1.0 / H
    stats[H:, 1] = 1.0 / H
    gamma2 = np.zeros((128, 128), dtype=np.float32)
    beta2 = np.zeros((128, 128), dtype=np.float32)
    for base in (0, 32, 64):
        gamma2[base, :H] = gamma
        gamma2[base + 1, H:] = gamma
        beta2[base, :H] = beta
        beta2[base + 1, H:] = beta
    w2col = np.zeros((128, 2), dtype=np.float32)
    w2col[:H, 0] = W2[:, 0]
    w2col[H:, 1] = W2[:, 0]
    iota = np.tile(np.arange(128, dtype=np.float32), (128, 1))
    beta2col = np.concatenate([beta, beta]).reshape(128, 1).astype(np.float32)
    return dict(
        w1blk=w1blk.astype(bf), b1col=b1col, beta2col=beta2col,
        stats_lhsT=stats.astype(bf), gamma2=gamma2.astype(bf),
        beta2=beta2.astype(bf), w2col=w2col.astype(bf),
        identd=np.eye(128, dtype=np.float32).astype(bf),
        iota_d=iota.astype(bf))


def build(cfg, meta, wvals):
    from concourse import bass, bacc, tile, mybir

    f32 = mybir.dt.float32
    bf16 = mybir.dt.bfloat16
    fp8 = mybir.dt.float8e4
    i16 = mybir.dt.int16
    AF = mybir.ActivationFunctionType
    ALU = mybir.AluOpType

    B, BH = cfg["B"], cfg["BH"]
    NBLK, CS, CB, SPLIT = cfg["NBLK"], cfg["CS"], cfg["CB"], cfg["SPLIT"]
    CPAD = cfg["CPAD"]
    TA, TAH = cfg["TA"], cfg["TAH"]
    TB, TBL = cfg["TB"], cfg["TBL"]
    GROUP = cfg["GROUP"]
    n_cores = cfg["n_cores"]
    schedA, schedB = meta["schedA"], meta["schedB"]
    KC = TG // 128                       # 8 chunks per gather call
    NTOKC = NBLK * B * 128               # tokens per core
    assert NTOKC % GROUP == 0

    nc = bacc.Bacc("TRN2", target_bir_lowering=False, debug=False,
                   num_devices=n_cores, num_swdge_queues=4)

    embA = nc.dram_tensor("embA", [128, TA // 128, BH], bf16,
                          kind="ExternalInput")
    indA = nc.dram_tensor("indA", [128, TA // 128, 128], fp8,
                          kind="ExternalInput")
    embN = nc.dram_tensor("embN", [128, NBLK, BH], bf16, kind="ExternalInput")
    gidxB = nc.dram_tensor("gidxB", [128, TB // 16], i16, kind="ExternalInput")
    indwB = nc.dram_tensor("indwB", [128, TB // 128, 128], fp8,
                           kind="ExternalInput")
    iota_d = nc.dram_tensor("iota_d", [128, 128], bf16, kind="ExternalInput")
    w1blk = nc.dram_tensor("w1blk", [128, 128], bf16, kind="ExternalInput")
    b1col = nc.dram_tensor("b1col", [128, 1], f32, kind="ExternalInput")
    stats_lhsT = nc.dram_tensor("stats_lhsT", [128, 2], bf16,
                                kind="ExternalInput")
    gamma2 = nc.dram_tensor("gamma2", [128, 128], bf16, kind="ExternalInput")
    beta2 = nc.dram_tensor("beta2", [128, 128], bf16, kind="ExternalInput")
    w2col = nc.dram_tensor("w2col", [128, 2], bf16, kind="ExternalInput")
    identd = nc.dram_tensor("identd", [128, 128], bf16, kind="ExternalInput")
    beta2col = nc.dram_tensor("beta2col", [128, 1], f32,
                              kind="ExternalInput")
    out = nc.dram_tensor("out", [NTOKC // GROUP, GROUP], f32,
                         kind="ExternalOutput")

    chslice = nc.dram_tensor("chslice", [CS[0] + CS[1], BH], fp8)
    tableL = nc.dram_tensor("tableL", [SPLIT, BH], fp8,
                            addr_space="Shared")
    tableH = nc.dram_tensor("tableH", [CPAD - SPLIT, BH], fp8,
                            addr_space="Shared")
    invn_tok = nc.dram_tensor("invn_tok", [128, NBLK], f32,
                              kind="ExternalInput")

    b2v = float(wvals["b2"])
    groups8 = [list(range(n_cores))]

    with tile.TileContext(nc) as tc:
        AKC = 2 * KC  # embA DMA chunk: 2048 rows
        with tc.tile_pool(name="pw", bufs=1) as pw:
            w1_s = pw.tile([128, 128], bf16)
            b1_s = pw.tile([128, 1], f32)
            st_s = pw.tile([128, 2], bf16)
            ga_s = pw.tile([128, 128], bf16)
            be_s = pw.tile([128, 128], bf16)
            w2_s = pw.tile([128, 2], bf16)
            ident = pw.tile([128, 128], bf16)
            ones2 = pw.tile([128, GROUP], bf16)
            epsc = pw.tile([128, 1], f32)
            beta_c = pw.tile([128, 1], f32)
            ib_t = pw.tile([128, TB // 16], i16)
            iota_s = pw.tile([128, 128], bf16)
            ivn_s = pw.tile([128, NBLK], f32)
            nc.scalar.dma_start(w1_s[:], w1blk[:, :])
            nc.scalar.dma_start(b1_s[:], b1col[:, :])
            nc.scalar.dma_start(st_s[:], stats_lhsT[:, :])
            nc.scalar.dma_start(ga_s[:], gamma2[:, :])
            nc.scalar.dma_start(be_s[:], beta2[:, :])
            nc.scalar.dma_start(w2_s[:], w2col[:, :])
            nc.scalar.dma_start(ident[:], identd[:, :])
            nc.scalar.dma_start(ib_t[:], gidxB[:, :])
            nc.scalar.dma_start(iota_s[:], iota_d[:, :])
            nc.scalar.dma_start(ivn_s[:], invn_tok[:, :])
            nc.vector.memset(ones2[:], 1.0)
            nc.vector.memset(epsc[:], LN_EPS)
            nc.scalar.dma_start(beta_c[:], beta2col[:, :])

            # ================= stage A (per half) =================
            ck_base = [0, TAH[0] // 128]
            for h2 in (0, 1):
                with tc.tile_pool(name=f"pAg{h2}", bufs=6) as pAg, \
                     tc.tile_pool(name=f"pAn{h2}", bufs=1) as pAn, \
                     tc.tile_pool(name=f"pAp{h2}", bufs=2,
                                  space="PSUM") as pAp:
                    sbt = pAn.tile([128, CB[h2], BH], fp8, tag="sbt")
                    ck_lo = ck_base[h2]
                    ck_hi = ck_lo + TAH[h2] // 128
                    state = dict(tile=None, ind=None, pend=[], prev=None)

                    def flushA():
                        pend = state["pend"]
                        if not pend:
                            return
                        j = state["prev"]
                        ps = pAp.tile([128, BH], f32, tag="psA")
                        for i, (tl, ind_t, kk) in enumerate(pend):
                            nc.tensor.matmul(ps[:], ind_t[:, kk, :],
                                             tl[:, kk, :], start=(i == 0),
                                             stop=(i == len(pend) - 1))
                        nc.scalar.copy(sbt[:, j:j + 1, :],
                                       ps[:].unsqueeze(1))

                    for ck in range(ck_lo, ck_hi):
                        _h, j = schedA[ck]
                        if (ck - ck_lo) % AKC == 0:
                            n = min(AKC, ck_hi - ck)
                            state["tile"] = pAg.tile([128, AKC, BH], bf16,
                                                     tag="gA", name="gA")
                            eng = [nc.scalar, nc.sync][(ck // AKC) % 2]
                            eng.dma_start(
                                state["tile"][:, :n, :],
                                bass.AP(embA, ck * BH,
                                        [[TA // 128 * BH, 128], [BH, n],
                                         [1, BH]]))
                            state["ind"] = pAg.tile([128, AKC, 128], fp8,
                                                    tag="indA", name="indAt")
                            eng2 = [nc.sync, nc.scalar][(ck // AKC) % 2]
                            eng2.dma_start(
                                state["ind"][:, :n, :],
                                bass.AP(indA, ck * 128,
                                        [[TA // 128 * 128, 128], [128, n],
                                         [1, 128]]))
                        if j < 0:
                            continue
                        if j != state["prev"]:
                            flushA()
                            state["pend"] = []
                            state["prev"] = j
                        state["pend"].append(
                            (state["tile"], state["ind"], (ck - ck_lo) % AKC))
                    flushA()
                    cbase = 0 if h2 == 0 else CS[0]
                    for j0 in range(0, CB[h2], 5):
                        j1 = min(j0 + 5, CB[h2])
                        nc.sync.dma_start(
                            bass.AP(chslice, (cbase + j0 * 128) * BH,
                                    [[BH, 128], [128 * BH, j1 - j0],
                                     [1, BH]]),
                            sbt[:, j0:j1, :])
                tab = tableL if h2 == 0 else tableH
                trows = SPLIT if h2 == 0 else CPAD - SPLIT
                nc.gpsimd.collective_compute(
                    "AllGather", mybir.AluOpType.bypass,
                    replica_groups=groups8,
                    ins=[bass.AP(chslice, (0 if h2 == 0 else CS[0]) * BH,
                                 [[1, CS[h2] * BH]]).opt()],
                    outs=[bass.AP(tab, 0, [[1, trows * BH]]).opt()])

            # ================= stage B =================
            capL = bass.AP(tableL, 0, [[BH, SPLIT], [1, BH]])
            capH = bass.AP(tableH, 0, [[BH, CPAD - SPLIT], [1, BH]])

            # chunk lists per (half, nb)
            blk_chunks = {0: {}, 1: {}}
            for ck, (h, nb) in enumerate(schedB):
                if nb >= 0:
                    blk_chunks[h].setdefault(nb, []).append(ck)

            gtiles = {}

            with tc.tile_pool(name="pm", bufs=2) as pm, \
                 tc.tile_pool(name="pBg", bufs=8) as pBg, \
                 tc.tile_pool(name="pctx", bufs=1) as pctx, \
                 tc.tile_pool(name="pp", bufs=2, space="PSUM") as pp, \
                 tc.tile_pool(name="pp1", bufs=1, space="PSUM") as pp1, \
                 tc.tile_pool(name="ppB", bufs=2, space="PSUM") as ppB:

                ctxL = pctx.tile([128, NBLK, BH], bf16)

                def get_gtile(call):
                    if call not in gtiles:
                        src = capL if call < TBL // TG else capH
                        t = pBg.tile([128, KC, BH], fp8, tag="gB", bufs=12,
                                     name="gB")
                        nc.gpsimd.dma_gather(
                            t[:], src,
                            ib_t[:, call * TG // 16:(call + 1) * TG // 16],
                            TG, TG, BH, queue_num=call % 4)
                        iw = pBg.tile([128, KC, 128], fp8, tag="iwB", bufs=12,
                                      name="iwB")
                        nc.sync.dma_start(
                            iw[:],
                            bass.AP(indwB, call * KC * 128,
                                    [[TB, 128], [128, KC], [1, 128]]))
                        gtiles[call] = (t, iw)
                    return gtiles[call]

                # ---- LOW pass: ctxL per block (incl. emb*deg via ident)
                lastL = {nb: max(cks) // KC
                         for nb, cks in blk_chunks[0].items()}
                ncall_L = TBL // TG
                nbdone = 0
                etL = {}
                NES = 32
                for cg in range(0, ncall_L, 8):
                    for call in range(cg, min(cg + 8, ncall_L)):
                        get_gtile(call)
                    cov = min(cg + 8, ncall_L) - 1
                    while nbdone < NBLK and lastL.get(nbdone, -1) <= cov:
                        nb = nbdone
                        es = nb // NES
                        if es not in etL:
                            ne = min(NES, NBLK - es * NES)
                            etl = pm.tile([128, NES, BH], bf16, tag="embL",
                                          bufs=2, name="etl")
                            nc.scalar.dma_start(
                                etl[:, :ne, :],
                                bass.AP(embN, es * NES * BH,
                                        [[NBLK * BH, 128], [BH, ne],
                                         [1, BH]]))
                            etL[es] = etl
                        cks = blk_chunks[0].get(nb, [])
                        ps = ppB.tile([128, BH], f32, tag="psB")
                        nc.tensor.matmul(ps[:], ident[:],
                                         etL[es][:, nb % NES, :],
                                         start=True, stop=False)
                        for i, ck in enumerate(cks):
                            tl, iw = get_gtile(ck // KC)
                            nc.tensor.matmul(ps[:], iw[:, ck % KC, :],
                                             tl[:, ck % KC, :],
                                             start=False,
                                             stop=(i == len(cks) - 1))
                        nc.scalar.copy(ctxL[:, nb:nb + 1, :],
                                       ps[:].unsqueeze(1))
                        nbdone += 1

                # ---- HIGH pass + MLP, slabs of SLABB blocks
                SLABB = 12
                lastH = {nb: (max(cks) - TBL // 128) // KC
                         for nb, cks in blk_chunks[1].items()}
                ncall_H = (TB - TBL) // TG
                gpc = GROUP // 128
                nb0 = 0
                issuedH = -1
                while nb0 < NBLK:
                    nbs = min(SLABB, NBLK - nb0)
                    t0 = nb0 * B * 128           # token base
                    xt = pm.tile([128, SLABB, BH], bf16, tag="xm", bufs=3)
                    jdone = 0
                    while jdone < nbs:
                        if issuedH < ncall_H - 1:
                            hi = min(issuedH + 8, ncall_H - 1)
                            for call in range(issuedH + 1, hi + 1):
                                get_gtile(ncall_L + call)
                            issuedH = hi
                        while jdone < nbs and \
                                lastH.get(nb0 + jdone, -1) <= issuedH:
                            j = jdone
                            nb = nb0 + j
                            cks = blk_chunks[1].get(nb, [])
                            ps = ppB.tile([128, BH], f32, tag="psB")
                            nc.tensor.matmul(ps[:], ident[:],
                                             ctxL[:, nb, :],
                                             start=True, stop=False)
                            for i, ck in enumerate(cks):
                                call = ncall_L + (ck - TBL // 128) // KC
                                tl, iw = get_gtile(call)
                                kk = (ck - TBL // 128) % KC
                                nc.tensor.matmul(ps[:], iw[:, kk, :],
                                                 tl[:, kk, :],
                                                 start=False,
                                                 stop=(i == len(cks) - 1))
                            nc.vector.tensor_scalar(
                                xt[:, j:j + 1, :], ps[:].unsqueeze(1),
                                ivn_s[:, nb:nb + 1], None,
                                ALU.mult, ALU.bypass)
                            jdone += 1

                    # ---- MLP on this slab
                    ntok = nbs * B * 128
                    npr = ntok // (2 * GROUP)
                    for pg0 in range(0, npr, 3):
                        prs = list(range(pg0, min(pg0 + 3, npr)))
                        pst8 = pp1.tile([128, GROUP], f32, tag="pst8")
                        psq8 = pp1.tile([128, GROUP], f32, tag="psq8")
                        h1s = {}
                        for jl, pr in enumerate(prs):
                            xT = pp.tile([128, GROUP], bf16, tag="mmp")
                            for jj in range(gpc):
                                c0 = 2 * (pr * gpc + jj)
                                nc.tensor.transpose(
                                    xT[:, jj * 128:(jj + 1) * 128],
                                    xt[:, c0 // B,
                                       (c0 % B) * H:(c0 % B) * H + 2 * H]
                                    .unsqueeze(1), ident[:])
                            xT_sb = pm.tile([128, GROUP], bf16, tag="xTsb",
                                            bufs=3)
                            nc.scalar.copy(xT_sb[:], xT[:])
                            ph = pp.tile([128, GROUP], f32, tag="mmp")
                            nc.tensor.matmul(ph[:], w1_s[:], xT_sb[:])
                            h1 = pm.tile([128, GROUP], bf16, tag="h1",
                                         bufs=6)
                            sq = pm.tile([128, GROUP], bf16, tag="sq",
                                         bufs=3)
                            nc.scalar.activation(h1[:], ph[:], AF.Identity,
                                                 bias=b1_s[:], scale=1.0)
                            nc.scalar.activation(sq[:], h1[:], AF.Square)
                            nc.tensor.matmul(
                                pst8[32 * jl:32 * jl + 2, :], st_s[:], h1[:])
                            nc.tensor.matmul(
                                psq8[32 * jl:32 * jl + 2, :], st_s[:], sq[:])
                            h1s[pr] = h1
                        nrow = 32 * (len(prs) - 1) + 2
                        sm8 = pm.tile([128, GROUP], f32, tag="sm8", bufs=2)
                        var8 = pm.tile([128, GROUP], f32, tag="var8", bufs=2)
                        sd8 = pm.tile([128, GROUP], f32, tag="sd8", bufs=2)
                        rstd8 = pm.tile([128, GROUP], f32, tag="rstd8",
                                        bufs=2)
                        rstd8_bf = pm.tile([128, GROUP], bf16, tag="rstd8b",
                                           bufs=2)
                        affr8 = pm.tile([128, GROUP], bf16, tag="affr8",
                                        bufs=2)
                        nc.scalar.copy(sm8[:nrow, :], pst8[:nrow, :])
                        nc.vector.scalar_tensor_tensor(
                            var8[:nrow, :], sm8[:nrow, :], -1.0,
                            sm8[:nrow, :], ALU.mult, ALU.mult)
                        nc.vector.scalar_tensor_tensor(
                            var8[:nrow, :], psq8[:nrow, :], 1.0,
                            var8[:nrow, :], ALU.mult, ALU.add)
                        nc.scalar.activation(sd8[:nrow, :], var8[:nrow, :],
                                             AF.Sqrt, bias=epsc[:nrow, :],
                                             scale=1.0)
                        nc.vector.reciprocal_approx_fast(rstd8[:nrow, :],
                                                         sd8[:nrow, :])
                        nc.scalar.copy(rstd8_bf[:nrow, :], rstd8[:nrow, :])
                        nc.vector.scalar_tensor_tensor(
                            affr8[:nrow, :], sm8[:nrow, :], -1.0,
                            rstd8[:nrow, :], ALU.mult, ALU.mult)
                        for jl, pr in enumerate(prs):
                            h1 = h1s[pr]
                            pscale = pp1.tile([128, GROUP], f32,
                                              tag="pscale", bufs=2)
                            poff = pp1.tile([128, GROUP], f32, tag="pscale",
                                            bufs=2, name="poff")
                            sl = slice(32 * jl, 32 * jl + 2)
                            nc.tensor.matmul(
                                pscale[:], ga_s[sl, :], rstd8_bf[sl, :])
                            nc.tensor.matmul(poff[:], ga_s[sl, :],
                                             affr8[sl, :])
                            t1t = pm.tile([128, GROUP], f32, tag="t1t",
                                          bufs=3)
                            h3 = pm.tile([128, GROUP], bf16, tag="h3",
                                         bufs=3)
                            nc.vector.tensor_mul(t1t[:], h1[:], pscale[:])
                            nc.vector.tensor_add(t1t[:], t1t[:], poff[:])
                            nc.scalar.activation(h3[:], t1t[:], AF.Relu,
                                                 bias=beta_c[:], scale=1.0)
                            pL2 = pp1.tile([2, GROUP], f32, tag="pscale", bufs=2,
                                           name="pL2")
                            nc.tensor.matmul(pL2[:], w2_s[:], h3[:])
                            lgs = pm.tile([2, GROUP], f32, tag="lgs",
                                          bufs=3)
                            nc.vector.tensor_copy(lgs[:], pL2[:])
                            nc.sync.dma_start(
                                bass.AP(out, t0 + pr * 2 * GROUP,
                                        [[GROUP, 2], [1, GROUP]]),
                                lgs[:])
                    nb0 += nbs

    nc.compile()
    return nc


def build_in_maps(cfg, inputs, preA, preB, wts):
    import ml_dtypes
    bf = ml_dtypes.bfloat16
    emb_full = np.asarray(inputs["embedding"], dtype=np.float32)
    B, N, NS, NBLK, BH, TA = (cfg["B"], cfg["N"], cfg["NS"], cfg["NBLK"],
                              cfg["BH"], cfg["TA"])
    # [N, B*H] view of the embedding
    embT = np.ascontiguousarray(emb_full.transpose(1, 0, 2).reshape(N, BH))
    in_maps = []
    for g in range(cfg["n_cores"]):
        dA, dB = preA[g], preB[g]
        embA_a = np.zeros((TA, BH), dtype=bf)
        valid = dA["rowsrcA"] >= 0
        src_nodes = dA["mnode"][dA["rowsrcA"][valid]]
        embA_a[valid] = (embT[src_nodes] *
                         dA["minvc"][dA["rowsrcA"][valid]][:, None]).astype(bf)
        embA_a = np.ascontiguousarray(
            embA_a.reshape(-1, 128, BH).transpose(1, 0, 2))
        embN_a = np.zeros((NS, BH), dtype=bf)
        n0 = g * NS
        n1 = min(n0 + NS, N)
        deg = preB[g]["deg"][:n1 - n0]
        embN_a[:n1 - n0] = (embT[n0:n1] * deg[:, None]).astype(bf)
        embN_a = np.ascontiguousarray(
            embN_a.reshape(-1, 128, BH).transpose(1, 0, 2))
        m = dict(embA=embA_a, indA=dA["indA"],
                 embN=embN_a, gidxB=dB["gidxB"], indwB=dB["indwB"],
                 invn_tok=dB["invn_tok"], **wts)
        in_maps.append(m)
    return in_maps


def assemble_out(cfg, results, b2v=0.0):
    B, N, NS, NBLK, GROUP = (cfg["B"], cfg["N"], cfg["NS"], cfg["NBLK"],
                             cfg["GROUP"])
    out = np.empty((B, N), dtype=np.float32)
    for g in range(cfg["n_cores"]):
        a = np.asarray(results[g]["out"]).reshape(-1, 2, 4, 128) + b2v
        toks = a.transpose(0, 2, 1, 3).reshape(-1)   # token-major
        # token t = (nb*B + b)*128 + p ; node = g*NS + nb*128 + p
        t = toks.reshape(NBLK, B, 128)
        n0 = g * NS
        n1 = min(n0 + NS, N)
        for b in range(B):
            out[b, n0:n1] = t[:, b, :].reshape(-1)[:n1 - n0]
    return out


def kernel(**inputs):
    emb = np.asarray(inputs["embedding"])
    B, N, _ = emb.shape
    C = int(inputs["num_classes"])
    E = len(np.asarray(inputs["n2c_row"]))
    cfg = make_cfg(B, N, C, E)
    preA, preB, meta = host_prep(cfg, inputs)
    wts = weight_tensors(inputs)
    wvals = dict(b2=float(np.asarray(inputs["b2"]).reshape(-1)[0]))
    nc = build(cfg, meta, wvals)
    in_maps = build_in_maps(cfg, inputs, preA, preB, wts)
    from concourse.bass_utils import run_bass_kernel_spmd
    res = run_bass_kernel_spmd(nc, in_maps,
                               core_ids=list(range(cfg["n_cores"])))
    return assemble_out(cfg, res.results, wvals["b2"])
